# revision 1
# baseline (speedup 1.0000x reference)
"""Two-layer GraphSAGE (mean aggregation) on 8 Trainium2 NeuronCores — v2.

Strategy (dst-sharded nodes/edges, replicated weights, AllGather for the
layer-2 halo exchange), tuned for the memory roofline:

  * All compute in bf16 (PE 4x faster than fp32; gather traffic halved);
    PSUM accumulation stays fp32.
  * Edges are packed on the host into (window, src-block) cells padded to
    128 so every gathered slab slice belongs to exactly one 128-node dst
    window. Gathers are batched per 7-window group (4 src blocks -> 4
    dma_gather calls per group instead of 4 per window).
  * The one-hot segment matrices for ALL slices of a window are built by a
    single DVE tensor_tensor(is_equal) over broadcast access patterns.
  * deg_inv is folded in via a host-tiled [128, nwin*128] bf16 table:
    mean^T = psum * dinv_tile (one DVE op, PSUM->SBUF, no transposes).
  * Layer 2 transforms first (p2 = h @ W2_l, padded to 128 cols so the
    bf16 gather element is 256B), AllGathers p2 in 4 chunks overlapped
    with phase 1, then gathers p2 rows by edge source.
  * Output y accumulates in a resident [64, nwin*128] bf16 tile; one DMA.

Self-contained: hardcodes problem shapes from the task spec.
"""

import numpy as np

IN_CH, HIDDEN, OUT_CH = 128, 128, 64
N_NODES, N_EDGES = 100000, 1600000
NCORES = 8
P = 128
L1_RANGE = 25000             # src rows per L1 gather block (int16 idx limit)
GRP = 7                      # windows per gather group


def _derive_cfg(n_nodes):
    shard = n_nodes // NCORES
    nwin = (shard + P - 1) // P
    ngrp = (nwin + GRP - 1) // GRP
    nchunk = 4 if nwin >= 4 else 1
    chunk_wins = (nwin + nchunk - 1) // nchunk
    chunk_rows = []
    for c in range(nchunk):
        lo = c * chunk_wins * P
        hi = min((c + 1) * chunk_wins * P, nwin * P)
        chunk_rows.append(max(hi - lo, 0))
    nblk1 = (n_nodes + L1_RANGE - 1) // L1_RANGE
    return dict(shard=shard, nwin=nwin, ngrp=ngrp, nchunk=nchunk,
                chunk_wins=chunk_wins, chunk_rows=chunk_rows, nblk1=nblk1)


def _pack(core, win, blk, loc, dstl, nwin, nblk, zero_rows):
    """Pack edges into per-(window, block) cells padded to 128.

    Cell order: (group, block, window-in-group)  [gather-call layout].
    Returns idx16 [NCORES, 16, sumT//16], dt [NCORES, 128, sumC] (999 pad),
    cell slice-offset table C0 [nwin, nblk], cell slice counts C [nwin, nblk].
    """
    ngrp = (nwin + GRP - 1) // GRP
    cnt = np.bincount((core * nwin + win) * nblk + blk,
                      minlength=NCORES * nwin * nblk
                      ).reshape(NCORES, nwin, nblk)
    T = cnt.max(axis=0)                       # [nwin, nblk]
    T = (T + P - 1) // P * P                  # pad cells to 128
    C = T // P                                # slices per cell

    # global cell order (g, b, w_in_g)
    order_cells = []
    for g in range(ngrp):
        ws = range(g * GRP, min((g + 1) * GRP, nwin))
        for b in range(nblk):
            for w in ws:
                order_cells.append((w, b))
    cell_rank = np.full((nwin, nblk), -1, np.int64)
    col = np.zeros((nwin, nblk), np.int64)    # start column (edge units)
    acc = 0
    for r, (w, b) in enumerate(order_cells):
        cell_rank[w, b] = r
        col[w, b] = acc
        acc += T[w, b]
    sumT = acc
    C0 = col // P                             # start slice index

    # per-edge destination position
    cellid = cell_rank[win, blk]              # [E]
    keys = core.astype(np.int64) * len(order_cells) + cellid
    order = np.argsort(keys, kind="stable")
    ks = keys[order]
    # rank within (core, cell)
    first = np.zeros(len(ks), np.int64)
    if len(ks):
        newseg = np.ones(len(ks), bool)
        newseg[1:] = ks[1:] != ks[:-1]
        seg_starts = np.flatnonzero(newseg)
        first[seg_starts] = 1
        idxs = np.arange(len(ks))
        starts_for = idxs[newseg][np.cumsum(newseg) - 1]
        rank = idxs - starts_for
    else:
        rank = first
    pos = col[win[order], blk[order]] + rank  # col within core's table

    idx_flat = np.empty((NCORES, sumT), np.int32)
    for b in range(nblk):
        bcols = np.zeros(sumT, bool)
        for w in range(nwin):
            if T[w, b]:
                bcols[col[w, b]:col[w, b] + T[w, b]] = True
        idx_flat[:, bcols] = zero_rows[b]
    dt_flat = np.full((NCORES, sumT), 999.0, np.float32)
    co = core[order]
    idx_flat[co, pos] = loc[order]
    dt_flat[co, pos] = dstl[order]

    # wrap idx into 16 partitions per gather call (call = (g, b) range)
    idx16 = np.empty((NCORES, 16, sumT // 16), np.int16)
    for g in range(ngrp):
        ws = list(range(g * GRP, min((g + 1) * GRP, nwin)))
        for b in range(nblk):
            a = int(col[ws[0], b])
            e = int(col[ws[-1], b] + T[ws[-1], b])
            if e == a:
                continue
            seg = idx_flat[:, a:e]
            idx16[:, :, a // 16:e // 16] = (
                seg.reshape(NCORES, (e - a) // 16, 16).transpose(0, 2, 1))
    dt = dt_flat.reshape(NCORES, sumT // P, P).transpose(0, 2, 1)
    return idx16, np.ascontiguousarray(dt), C0, C, col, T, sumT


def _preprocess(x, edge_index, cfg):
    import ml_dtypes
    bf16 = ml_dtypes.bfloat16
    n = x.shape[0]
    shard, nwin = cfg["shard"], cfg["nwin"]
    nchunk, chunk_wins = cfg["nchunk"], cfg["chunk_wins"]
    chunk_rows, nblk1 = cfg["chunk_rows"], cfg["nblk1"]

    src = np.asarray(edge_index[0], dtype=np.int64)
    dst = np.asarray(edge_index[1], dtype=np.int64)
    deg = np.bincount(dst, minlength=n).astype(np.float32)
    deg_inv = np.where(deg > 0, np.float32(1.0) / np.maximum(deg, 1.0),
                       0.0).astype(np.float32)

    core = (dst // shard).astype(np.int64)
    local = dst - core * shard
    win = local // P
    dstl = (local % P).astype(np.int32)

    # ---- L1 cells
    blk1 = np.minimum(src // L1_RANGE, nblk1 - 1)
    loc1 = (src - blk1 * L1_RANGE).astype(np.int32)
    blk1_rows = [min(L1_RANGE, n - q * L1_RANGE) for q in range(nblk1)]
    zr1 = blk1_rows                    # zero row index per block
    idx1, dt1, C01, C1, col1, T1, sumT1 = _pack(
        core, win, blk1, loc1, dstl, nwin, nblk1, zr1)

    # ---- L2 cells (blocks = AllGather chunks)
    csz = chunk_wins * P
    blk2 = np.minimum((src % shard) // csz, nchunk - 1)
    cr = np.array(chunk_rows)
    loc2 = ((src // shard) * cr[blk2] + (src % shard) - blk2 * csz
            ).astype(np.int32)
    zr2 = [NCORES * r for r in chunk_rows]
    idx2, dt2, C02, C2, col2, T2, sumT2 = _pack(
        core, win, blk2, loc2, dstl, nwin, nchunk, zr2)

    # ---- x tables
    xb = x.astype(bf16)
    xblocks = []
    for q in range(nblk1):
        blkx = xb[q * L1_RANGE:q * L1_RANGE + blk1_rows[q]]
        xblocks.append(np.concatenate(
            [blkx, np.zeros((1, x.shape[1]), bf16)]))
    xdev = np.ascontiguousarray(np.concatenate(xblocks, axis=0))
    l1_base = np.concatenate(
        [[0], np.cumsum([b.shape[0] for b in xblocks])])[:-1]

    xts, dinvts = [], []
    for ci in range(NCORES):
        xs = x[ci * shard:(ci + 1) * shard]
        pad = nwin * P - shard
        xts.append(np.ascontiguousarray(np.concatenate(
            [xs, np.zeros((pad, x.shape[1]), np.float32)]).T).astype(bf16))
        dv = np.concatenate([deg_inv[ci * shard:(ci + 1) * shard],
                             np.zeros(pad, np.float32)])
        dinvts.append(np.ascontiguousarray(
            np.tile(dv[None, :], (P, 1))).astype(bf16))

    meta = dict(C01=C01, C1=C1, col1=col1, T1=T1, sumT1=sumT1,
                C02=C02, C2=C2, col2=col2, T2=T2, sumT2=sumT2,
                l1_base=l1_base, blk1_rows=blk1_rows)
    data = dict(xdev=xdev, idx1=idx1, dt1=dt1, idx2=idx2, dt2=dt2,
                xts=xts, dinvts=dinvts)
    return meta, data


# ---------------------------------------------------------------- builder

def _build(cfg, meta, repeat=1, debug=False, maxg=512, ablate=(),
           nq=4, single_packet=False, slab_bufs=3):
    import concourse.bacc as bacc
    import concourse.mybir as mybir
    import concourse.tile as tile

    f32 = mybir.dt.float32
    bf16 = mybir.dt.bfloat16
    i16 = mybir.dt.int16
    shard, nwin, ngrp = cfg["shard"], cfg["nwin"], cfg["ngrp"]
    nchunk, chunk_wins = cfg["nchunk"], cfg["chunk_wins"]
    chunk_rows, nblk1 = cfg["chunk_rows"], cfg["nblk1"]
    C01, C1, col1, T1, sumT1 = (meta[k] for k in
                                ("C01", "C1", "col1", "T1", "sumT1"))
    C02, C2, col2, T2, sumT2 = (meta[k] for k in
                                ("C02", "C2", "col2", "T2", "sumT2"))
    l1_base, blk1_rows = meta["l1_base"], meta["blk1_rows"]
    xdev_rows = int(l1_base[-1] + blk1_rows[-1] + 1)

    p2_off = np.concatenate(
        [[0], np.cumsum([NCORES * r + 1 for r in chunk_rows])])
    p2_rows = int(p2_off[-1])

    # group extents
    def grp_windows(g):
        return list(range(g * GRP, min((g + 1) * GRP, nwin)))

    def grp_cols(col, T, g, nblk):
        ws = grp_windows(g)
        a = int(col[ws[0], 0])
        last_b = nblk - 1
        e = int(col[ws[-1], last_b] + T[ws[-1], last_b])
        return a, e

    slab1_max = max(grp_cols(col1, T1, g, nblk1)[1] -
                    grp_cols(col1, T1, g, nblk1)[0] for g in range(ngrp))
    slab2_max = max(grp_cols(col2, T2, g, nchunk)[1] -
                    grp_cols(col2, T2, g, nchunk)[0] for g in range(ngrp))
    slab_max = max(slab1_max, slab2_max)
    s1w_max = int(C1.sum(axis=1).max())
    s2w_max = int(C2.sum(axis=1).max())
    sw_max = max(s1w_max, s2w_max)

    nc = bacc.Bacc(num_swdge_queues=nq)
    dp = nc.declare_dram_parameter
    xdev = dp("xdev", [xdev_rows, IN_CH], bf16, isOutput=False)
    xt = dp("xt", [P, nwin * P], bf16, isOutput=False)
    dinvt = dp("dinvt", [P, nwin * P], bf16, isOutput=False)
    idx1 = dp("idx1", [P, sumT1 // 16], i16, isOutput=False)
    dt1 = dp("dt1", [P, sumT1 // P], f32, isOutput=False)
    idx2 = dp("idx2", [P, sumT2 // 16], i16, isOutput=False)
    dt2 = dp("dt2", [P, sumT2 // P], f32, isOutput=False)
    w1l = dp("w1l", [IN_CH, HIDDEN], bf16, isOutput=False)
    w1r = dp("w1r", [IN_CH, HIDDEN], bf16, isOutput=False)
    w2lp = dp("w2lp", [HIDDEN, P], bf16, isOutput=False)   # zero-padded cols
    w2r = dp("w2r", [HIDDEN, OUT_CH], bf16, isOutput=False)
    b1c = dp("b1c", [P, 1], f32, isOutput=False)
    b2c = dp("b2c", [P, 1], f32, isOutput=False)
    iota = dp("iota", [P, P], f32, isOutput=False)
    y = dp("y", [OUT_CH, nwin * P], bf16, isOutput=True)
    if debug:
        dbg_t1t = dp("dbg_t1t", [P, nwin * P], bf16, isOutput=True)
        dbg_ht = dp("dbg_ht", [P, nwin * P], bf16, isOutput=True)
        dbg_dinv = dp("dbg_dinv", [P, nwin * P], bf16, isOutput=True)
        dbg_raw = dp("dbg_raw", [P, nwin * P], f32, isOutput=True)
        dbg_gat = dp("dbg_gat", [P, 4096], bf16, isOutput=True)
        dbg_m = dp("dbg_m", [P, 4096], bf16, isOutput=True)

    p2_full = nc.dram_tensor("p2_full", [p2_rows, P], bf16,
                             addr_space="Shared")
    p2_loc = nc.dram_tensor("p2_loc", [p2_rows, P], bf16)

    with tile.TileContext(nc) as tc:
        with (
            tc.tile_pool(name="const", bufs=1) as cb,
            tc.tile_pool(name="slab", bufs=slab_bufs) as slb,
            tc.tile_pool(name="sb", bufs=3) as sb,
            tc.tile_pool(name="mt", bufs=2) as mtp,
            tc.tile_pool(name="ps", bufs=2, space="PSUM") as ps,
            tc.tile_pool(name="psb", bufs=2, space="PSUM") as psb,
            tc.tile_pool(name="dram", bufs=1, space="DRAM") as dr,
        ):
            def cload(param, shape, dtype, tag):
                t = cb.tile(shape, dtype, tag=tag)
                nc.sync.dma_start(out=t[:], in_=param[:])
                return t

            iota_t = cload(iota, [P, P], f32, "c_iota")
            w1l_t = cload(w1l, [IN_CH, HIDDEN], bf16, "c_w1l")
            w1r_t = cload(w1r, [IN_CH, HIDDEN], bf16, "c_w1r")
            w2lp_t = cload(w2lp, [HIDDEN, P], bf16, "c_w2lp")
            w2r_t = cload(w2r, [HIDDEN, OUT_CH], bf16, "c_w2r")
            b1_t = cload(b1c, [P, 1], f32, "c_b1")
            b2_t = cload(b2c, [P, 1], f32, "c_b2")
            xt_t = cload(xt, [P, nwin * P], bf16, "c_xt")
            dinv_t = cload(dinvt, [P, nwin * P], bf16, "c_dinv")
            y_t = cb.tile([OUT_CH, nwin * P], bf16, tag="c_y")
            zrow_t = cb.tile([P, P], bf16, tag="c_zrow")
            nc.vector.memset(zrow_t[:], 0.0)

            p2c = []
            for c in range(nchunk):
                p2c_t = dr.tile([max(chunk_rows[c], 1), P], bf16,
                                tag=f"p2c{c}")
                p2c.append(p2c_t)
            for c in range(nchunk):
                zr = int(p2_off[c] + NCORES * chunk_rows[c])
                nc.sync.dma_start(out=p2_full[zr:zr + 1, :], in_=zrow_t[:1, :])
                nc.sync.dma_start(out=p2_loc[zr:zr + 1, :], in_=zrow_t[:1, :])

            relu = mybir.ActivationFunctionType.Relu
            copyf = mybir.ActivationFunctionType.Copy
            iseq = mybir.AluOpType.is_equal
            mult = mybir.AluOpType.mult
            add = mybir.AluOpType.add

            for _rep in range(repeat):
                # ---------------- phase 1 ----------------
                for g in range(ngrp):
                    ws = grp_windows(g)
                    a, e = grp_cols(col1, T1, g, nblk1)
                    it = sb.tile([P, slab_max // 16], i16, tag="it")
                    nc.sync.dma_start(out=it[:, :(e - a) // 16],
                                      in_=idx1[:, a // 16:e // 16])
                    dtt = sb.tile([P, slab_max // P], f32, tag="dtt")
                    nc.sync.dma_start(out=dtt[:, :(e - a) // P],
                                      in_=dt1[:, a // P:e // P])
                    gat = slb.tile([P, slab_max], bf16, tag="g")
                    for b in range(nblk1):
                        blo = int(l1_base[b])
                        nrows = blk1_rows[b] + 1
                        # split call ranges at cell boundaries, <= maxg idxs
                        spans = []
                        for w in ws:
                            t = int(T1[w, b])
                            if t == 0:
                                continue
                            c0 = int(col1[w, b])
                            if spans and spans[-1][1] - spans[-1][0] + t <= maxg:
                                spans[-1] = (spans[-1][0], c0 + t)
                            else:
                                spans.append((c0, c0 + t))
                        for si, (ca, ce) in enumerate(spans):
                            t_q = ce - ca
                            nc.gpsimd.dma_gather(
                                out_ap=gat[:, ca - a:ce - a]
                                .rearrange("p (c e) -> p c e", e=IN_CH),
                                in_ap=xdev[blo:blo + nrows, :],
                                idxs_ap=it[:, (ca - a) // 16:(ce - a) // 16],
                                num_idxs=t_q,
                                num_idxs_reg=t_q,
                                elem_size=IN_CH,
                                single_packet=single_packet,
                                queue_num=(b + si) % nq,
                            )
                    if debug and g == 0:
                        nc.sync.dma_start(out=dbg_gat[:, :4096],
                                          in_=gat[:, :4096])
                    for w in ws:
                        s1w = int(C1[w].sum())
                        n_w = min(shard - w * P, P)
                        if s1w > 0:
                            # one-hot bank for all slices of this window
                            m_t = mtp.tile([P, sw_max * P], bf16, tag="m")
                            k = 0
                            for b in range(nblk1):
                                c0 = int(C01[w, b]) - a // P
                                for sß in range(int(C1[w, b])):
                                    nc.vector.tensor_scalar(
                                        out=m_t[:, k * P:(k + 1) * P],
                                        in0=iota_t[:],
                                        scalar1=dtt[:, c0 + sß:c0 + sß + 1],
                                        scalar2=None, op0=iseq)
                                    k += 1
                            psum1 = ps.tile([P, P], f32, tag="psA",
                                            space="PSUM")
                            k = 0
                            for b in range(nblk1):
                                c0 = int(C01[w, b])
                                for s in range(int(C1[w, b])):
                                    sl = (c0 + s) * P - a
                                    nc.tensor.matmul(
                                        out=psum1[:],
                                        lhsT=gat[:, sl:sl + P],
                                        rhs=m_t[:, k * P:(k + 1) * P],
                                        start=(k == 0), stop=(k == s1w - 1))
                                    k += 1
                            if debug and w == 0:
                                nc.sync.dma_start(
                                    out=dbg_m[:, :min(s1w * P, 4096)],
                                    in_=m_t[:, :min(s1w * P, 4096)])
                            t1t = sb.tile([P, P], bf16, tag="t1t")
                            nc.vector.tensor_tensor(
                                out=t1t[:], in0=psum1[:],
                                in1=dinv_t[:, w * P:(w + 1) * P], op=mult)
                        else:
                            t1t = sb.tile([P, P], bf16, tag="t1t")
                            nc.vector.memset(t1t[:], 0.0)
                        psum2 = psb.tile([P, P], f32, tag="ps2", space="PSUM")
                        nc.tensor.matmul(out=psum2[:], lhsT=w1l_t[:],
                                         rhs=t1t[:], start=True, stop=False)
                        nc.tensor.matmul(out=psum2[:], lhsT=w1r_t[:],
                                         rhs=xt_t[:, w * P:(w + 1) * P],
                                         start=False, stop=True)
                        ht = sb.tile([P, P], bf16, tag="ht")
                        nc.scalar.activation(out=ht[:], in_=psum2[:],
                                             func=relu, bias=b1_t[:, :1],
                                             scale=1.0)
                        if debug:
                            nc.sync.dma_start(
                                out=dbg_t1t[:, w * P:(w + 1) * P], in_=t1t[:])
                            nc.sync.dma_start(
                                out=dbg_ht[:, w * P:(w + 1) * P], in_=ht[:])
                            nc.sync.dma_start(
                                out=dbg_dinv[:, w * P:(w + 1) * P],
                                in_=dinv_t[:, w * P:(w + 1) * P])
                            if s1w > 0:
                                raw = sb.tile([P, P], f32, tag="dbgraw")
                                nc.vector.tensor_copy(out=raw[:], in_=psum1[:])
                                nc.sync.dma_start(
                                    out=dbg_raw[:, w * P:(w + 1) * P],
                                    in_=raw[:])
                        # p2 rows [n, 128] = ht^T @ W2l_pad
                        psum3 = psb.tile([P, P], f32, tag="ps3", space="PSUM")
                        nc.tensor.matmul(out=psum3[:], lhsT=ht[:],
                                         rhs=w2lp_t[:], start=True, stop=True)
                        p2sb = sb.tile([P, P], bf16, tag="p2sb")
                        nc.scalar.activation(out=p2sb[:], in_=psum3[:],
                                             func=copyf)
                        c = min(w // chunk_wins, nchunk - 1)
                        r0 = w * P - c * chunk_wins * P
                        nc.sync.dma_start(out=p2c[c][r0:r0 + n_w, :],
                                          in_=p2sb[:n_w, :])
                        if n_w < P:
                            nc.sync.dma_start(
                                out=p2c[c][r0 + n_w:r0 + P, :],
                                in_=zrow_t[:P - n_w, :])
                        # r2 = (h @ W2r)^T + b2 -> y accumulator
                        psum4 = psb.tile([OUT_CH, P], f32, tag="ps4",
                                         space="PSUM")
                        nc.tensor.matmul(out=psum4[:], lhsT=w2r_t[:],
                                         rhs=ht[:], start=True, stop=True)
                        nc.vector.tensor_scalar(
                            out=y_t[:, w * P:(w + 1) * P], in0=psum4[:],
                            scalar1=b2_t[:OUT_CH, :1], scalar2=None, op0=add)
                        if (w + 1) % chunk_wins == 0 or w == nwin - 1:
                            c_done = (min(w // chunk_wins, nchunk - 1))
                            if (w + 1) % chunk_wins == 0 or w == nwin - 1:
                                off = int(p2_off[c_done])
                                rows = NCORES * chunk_rows[c_done]
                                if "noag" in ablate:
                                    pass
                                elif NCORES > 1:
                                    nc.gpsimd.collective_compute(
                                        "AllGather",
                                        mybir.AluOpType.bypass,
                                        replica_groups=[list(range(NCORES))],
                                        ins=[p2c[c_done].opt()],
                                        outs=[p2_full[off:off + rows, :]],
                                    )
                                else:
                                    nc.sync.dma_start(
                                        out=p2_full[off:off + rows, :],
                                        in_=p2c[c_done][:, :])
                                if "noag" not in ablate:
                                    nc.sync.dma_start(
                                        out=p2_loc[off:off + rows, :],
                                        in_=p2_full[off:off + rows, :])

                # ---------------- phase 2 ----------------
                for g in range(ngrp if "nop2" not in ablate else 0):
                    ws = grp_windows(g)
                    a, e = grp_cols(col2, T2, g, nchunk)
                    it = sb.tile([P, slab_max // 16], i16, tag="it")
                    nc.sync.dma_start(out=it[:, :(e - a) // 16],
                                      in_=idx2[:, a // 16:e // 16])
                    dtt = sb.tile([P, slab_max // P], f32, tag="dtt")
                    nc.sync.dma_start(out=dtt[:, :(e - a) // P],
                                      in_=dt2[:, a // P:e // P])
                    gat = slb.tile([P, slab_max], bf16, tag="g")
                    for b in range(nchunk):
                        off = int(p2_off[b])
                        nrows = NCORES * chunk_rows[b] + 1
                        spans = []
                        for w in ws:
                            t = int(T2[w, b])
                            if t == 0:
                                continue
                            c0 = int(col2[w, b])
                            if spans and spans[-1][1] - spans[-1][0] + t <= maxg:
                                spans[-1] = (spans[-1][0], c0 + t)
                            else:
                                spans.append((c0, c0 + t))
                        for si, (ca, ce) in enumerate(spans):
                            t_q = ce - ca
                            nc.gpsimd.dma_gather(
                                out_ap=gat[:, ca - a:ce - a]
                                .rearrange("p (c e) -> p c e", e=P),
                                in_ap=p2_loc[off:off + nrows, :],
                                idxs_ap=it[:, (ca - a) // 16:(ce - a) // 16],
                                num_idxs=t_q,
                                num_idxs_reg=t_q,
                                elem_size=P,
                                single_packet=single_packet,
                                queue_num=(b + si) % nq,
                            )
                    for w in ws:
                        s2w = int(C2[w].sum())
                        if s2w == 0:
                            continue
                        m_t = mtp.tile([P, sw_max * P], bf16, tag="m")
                        k = 0
                        for b in range(nchunk):
                            c0 = int(C02[w, b]) - a // P
                            for sß in range(int(C2[w, b])):
                                nc.vector.tensor_scalar(
                                    out=m_t[:, k * P:(k + 1) * P],
                                    in0=iota_t[:],
                                    scalar1=dtt[:, c0 + sß:c0 + sß + 1],
                                    scalar2=None, op0=iseq)
                                k += 1
                        psum5f = ps.tile([P, P], f32, tag="psA",
                                         space="PSUM")
                        psum5 = psum5f[:OUT_CH, :]
                        k = 0
                        for b in range(nchunk):
                            c0 = int(C02[w, b])
                            for s in range(int(C2[w, b])):
                                sl = (c0 + s) * P - a
                                nc.tensor.matmul(
                                    out=psum5,
                                    lhsT=gat[:, sl:sl + OUT_CH],
                                    rhs=m_t[:, k * P:(k + 1) * P],
                                    start=(k == 0), stop=(k == s2w - 1))
                                k += 1
                        agg2 = sb.tile([OUT_CH, P], bf16, tag="agg2")
                        nc.vector.tensor_tensor(
                            out=agg2[:], in0=psum5,
                            in1=dinv_t[:OUT_CH, w * P:(w + 1) * P], op=mult)
                        nc.vector.tensor_tensor(
                            out=y_t[:, w * P:(w + 1) * P], in0=agg2[:],
                            in1=y_t[:, w * P:(w + 1) * P], op=add)

            nc.sync.dma_start(out=y[:, :], in_=y_t[:, :])

    nc.compile()
    return nc


# ---------------------------------------------------------------- entry

_CACHE = {}


def _meta_sig(meta):
    return (int(meta["sumT1"]), int(meta["sumT2"]))


def _make_inmaps(inputs, meta, data):
    import ml_dtypes
    bf16 = ml_dtypes.bfloat16
    iota_v = np.tile(np.arange(P, dtype=np.float32), (P, 1))
    w2l = np.asarray(inputs["W2_l"], np.float32)
    w2lp = np.concatenate(
        [w2l, np.zeros((HIDDEN, P - OUT_CH), np.float32)], axis=1)
    b2 = np.asarray(inputs["b2"], np.float32)
    common = dict(
        xdev=data["xdev"],
        w1l=np.asarray(inputs["W1_l"], np.float32).astype(bf16),
        w1r=np.asarray(inputs["W1_r"], np.float32).astype(bf16),
        w2lp=w2lp.astype(bf16),
        w2r=np.asarray(inputs["W2_r"], np.float32).astype(bf16),
        b1c=np.asarray(inputs["b1"], np.float32).reshape(P, 1),
        b2c=np.concatenate([b2, np.zeros(P - OUT_CH, np.float32)]
                           ).reshape(P, 1),
        iota=iota_v,
    )
    in_maps = []
    for ci in range(NCORES):
        m = dict(common)
        m["xt"] = data["xts"][ci]
        m["dinvt"] = data["dinvts"][ci]
        m["idx1"] = np.tile(data["idx1"][ci], (8, 1))
        m["dt1"] = data["dt1"][ci]
        m["idx2"] = np.tile(data["idx2"][ci], (8, 1))
        m["dt2"] = data["dt2"][ci]
        in_maps.append(m)
    return in_maps


def kernel(x, edge_index, W1_l, W1_r, b1, W2_l, W2_r, b2):
    x = np.asarray(x, dtype=np.float32)
    edge_index = np.asarray(edge_index)
    cfg = _derive_cfg(x.shape[0])
    meta, data = _preprocess(x, edge_index, cfg)

    key = (x.shape, edge_index.shape)
    if key in _CACHE and _CACHE[key][1] == _meta_sig(meta):
        nc = _CACHE[key][0]
    else:
        nc = _build(cfg, meta)
        _CACHE[key] = (nc, _meta_sig(meta))

    in_maps = _make_inmaps(
        dict(W1_l=W1_l, W1_r=W1_r, b1=b1, W2_l=W2_l, W2_r=W2_r, b2=b2),
        meta, data)

    from concourse.bass_utils import run_bass_kernel_spmd
    r = run_bass_kernel_spmd(nc, in_maps, core_ids=list(range(NCORES)))
    shard = cfg["shard"]
    out = np.concatenate(
        [r.results[c]["y"].astype(np.float32).T[:shard]
         for c in range(NCORES)], axis=0)
    return np.ascontiguousarray(out, dtype=np.float32)



# revision 28
# speedup vs baseline: 1.5822x; 1.5822x over previous
"""Two-layer GraphSAGE (mean aggregation) on 8 Trainium2 NeuronCores.

Default strategy ("ag"): dst-sharded nodes/edges, replicated weights,
AllGather for the layer-2 halo exchange, tuned for the memory roofline:

  * All compute in bf16 (PSUM accumulation stays fp32); dst-position and
    iota tables in bf16 too (2x DVE throughput for the one-hot builds).
  * Edges are packed on the host into (window, src-block) cells padded to
    128 so every gathered slab slice belongs to exactly one 128-node dst
    window. Gather calls merge adjacent cells up to maxg=2048 indices
    (SWDGE desc-gen has ~1us fixed cost per call).
  * Cell-pad gather slots reference the cell's last real row, NOT a shared
    zero row: their values are killed by the all-zero one-hot column, and
    a shared row would serialize ~100k pad descriptors on one HBM bank
    (measured ~2.5 ms penalty before this fix).
  * Per-cell one-hot segment matrices are built by one DVE
    tensor_tensor(is_equal) over stride-0 broadcast access patterns.
  * deg_inv is folded in via a host-tiled [128, nwin*128] bf16 table:
    mean^T = psum * dinv_tile (one DVE op, PSUM->SBUF, no transposes).
  * Layer 2 transforms first (p2 = h @ W2_l, padded to 128 cols so the
    bf16 gather element is 256B), AllGathers p2 in 4 chunks overlapped
    with phase 1, then gathers p2 rows directly from the Shared AllGather
    output (no local copy).
  * Output y accumulates in a resident [64, nwin*128] bf16 tile; one DMA.

An alternative "rs" strategy (src-sharded layer 2: gather from the core's
own 3.2 MB p2 table, ch-major partial tables, one ReduceScatter) is kept
in _preprocess_rs/_build_rs; it measured ~3% slower on HW.

Self-contained: hardcodes problem shapes from the task spec.
"""

import numpy as np

IN_CH, HIDDEN, OUT_CH = 128, 128, 64
N_NODES, N_EDGES = 100000, 1600000
NCORES = 8
P = 128
L1_RANGE = 25000             # src rows per L1 gather block (int16 idx limit)
GRP = 7                      # windows per gather group


def _derive_cfg(n_nodes, nchunk=None):
    shard = n_nodes // NCORES
    nwin = (shard + P - 1) // P
    ngrp = (nwin + GRP - 1) // GRP
    if nchunk is None:
        nchunk = 4 if nwin >= 4 else 1
    chunk_wins = (nwin + nchunk - 1) // nchunk
    chunk_rows = []
    for c in range(nchunk):
        lo = c * chunk_wins * P
        hi = min((c + 1) * chunk_wins * P, nwin * P)
        chunk_rows.append(max(hi - lo, 0))
    nblk1 = (n_nodes + L1_RANGE - 1) // L1_RANGE
    return dict(shard=shard, nwin=nwin, ngrp=ngrp, nchunk=nchunk,
                chunk_wins=chunk_wins, chunk_rows=chunk_rows, nblk1=nblk1)


def _pack(core, win, blk, loc, dstl, nwin, nblk, zero_rows):
    """Pack edges into per-(window, block) cells padded to 128.

    Cell order: (group, block, window-in-group)  [gather-call layout].
    Returns idx16 [NCORES, 16, sumT//16], dt [NCORES, 128, sumC] (999 pad),
    cell slice-offset table C0 [nwin, nblk], cell slice counts C [nwin, nblk].
    """
    ngrp = (nwin + GRP - 1) // GRP
    cnt = np.bincount((core * nwin + win) * nblk + blk,
                      minlength=NCORES * nwin * nblk
                      ).reshape(NCORES, nwin, nblk)
    T = cnt.max(axis=0)                       # [nwin, nblk]
    T = (T + P - 1) // P * P                  # pad cells to 128
    C = T // P                                # slices per cell

    # global cell order (g, b, w_in_g)
    order_cells = []
    for g in range(ngrp):
        ws = range(g * GRP, min((g + 1) * GRP, nwin))
        for b in range(nblk):
            for w in ws:
                order_cells.append((w, b))
    cell_rank = np.full((nwin, nblk), -1, np.int64)
    col = np.zeros((nwin, nblk), np.int64)    # start column (edge units)
    acc = 0
    for r, (w, b) in enumerate(order_cells):
        cell_rank[w, b] = r
        col[w, b] = acc
        acc += T[w, b]
    sumT = acc
    C0 = col // P                             # start slice index

    # per-edge destination position
    cellid = cell_rank[win, blk]              # [E]
    keys = core.astype(np.int64) * len(order_cells) + cellid
    order = np.argsort(keys, kind="stable")
    ks = keys[order]
    # rank within (core, cell)
    first = np.zeros(len(ks), np.int64)
    if len(ks):
        newseg = np.ones(len(ks), bool)
        newseg[1:] = ks[1:] != ks[:-1]
        seg_starts = np.flatnonzero(newseg)
        first[seg_starts] = 1
        idxs = np.arange(len(ks))
        starts_for = idxs[newseg][np.cumsum(newseg) - 1]
        rank = idxs - starts_for
    else:
        rank = first
    pos = col[win[order], blk[order]] + rank  # col within core's table

    # Pad slots must reference SOME valid row (their one-hot column is
    # all-zero, so the value is never used). Use the cell's last real row
    # per core (row-buffer hit on HBM) — a shared zero row would turn the
    # pad descriptors into a single-bank hotspot.
    co = core[order]
    fill = np.empty((NCORES, nwin, nblk), np.int64)
    for b in range(nblk):
        fill[:, :, b] = (np.arange(nwin)[None, :] * 997) % zero_rows[b]
    fill[co, win[order], blk[order]] = loc[order]   # last write wins
    idx_flat = np.empty((NCORES, sumT), np.int32)
    for b in range(nblk):
        for w in range(nwin):
            if T[w, b]:
                idx_flat[:, col[w, b]:col[w, b] + T[w, b]] = \
                    fill[:, w, b][:, None]
    dt_flat = np.full((NCORES, sumT), 999.0, np.float32)
    idx_flat[co, pos] = loc[order]
    dt_flat[co, pos] = dstl[order]

    # wrap idx into 16 partitions per gather call (call = (g, b) range)
    idx16 = np.empty((NCORES, 16, sumT // 16), np.int16)
    for g in range(ngrp):
        ws = list(range(g * GRP, min((g + 1) * GRP, nwin)))
        for b in range(nblk):
            a = int(col[ws[0], b])
            e = int(col[ws[-1], b] + T[ws[-1], b])
            if e == a:
                continue
            seg = idx_flat[:, a:e]
            idx16[:, :, a // 16:e // 16] = (
                seg.reshape(NCORES, (e - a) // 16, 16).transpose(0, 2, 1))
    dt = dt_flat.reshape(NCORES, sumT // P, P).transpose(0, 2, 1)
    return idx16, np.ascontiguousarray(dt), C0, C, col, T, sumT


def _preprocess(x, edge_index, cfg):
    import ml_dtypes
    bf16 = ml_dtypes.bfloat16
    n = x.shape[0]
    shard, nwin = cfg["shard"], cfg["nwin"]
    nchunk, chunk_wins = cfg["nchunk"], cfg["chunk_wins"]
    chunk_rows, nblk1 = cfg["chunk_rows"], cfg["nblk1"]

    src = np.asarray(edge_index[0], dtype=np.int64)
    dst = np.asarray(edge_index[1], dtype=np.int64)
    deg = np.bincount(dst, minlength=n).astype(np.float32)
    deg_inv = np.where(deg > 0, np.float32(1.0) / np.maximum(deg, 1.0),
                       0.0).astype(np.float32)

    core = (dst // shard).astype(np.int64)
    local = dst - core * shard
    win = local // P
    dstl = (local % P).astype(np.int32)

    # ---- L1 cells
    blk1 = np.minimum(src // L1_RANGE, nblk1 - 1)
    loc1 = (src - blk1 * L1_RANGE).astype(np.int32)
    blk1_rows = [min(L1_RANGE, n - q * L1_RANGE) for q in range(nblk1)]
    zr1 = blk1_rows                    # zero row index per block
    idx1, dt1, C01, C1, col1, T1, sumT1 = _pack(
        core, win, blk1, loc1, dstl, nwin, nblk1, zr1)

    # ---- L2 cells (blocks = AllGather chunks)
    csz = chunk_wins * P
    blk2 = np.minimum((src % shard) // csz, nchunk - 1)
    cr = np.array(chunk_rows)
    loc2 = ((src // shard) * cr[blk2] + (src % shard) - blk2 * csz
            ).astype(np.int32)
    zr2 = [NCORES * r for r in chunk_rows]
    idx2, dt2, C02, C2, col2, T2, sumT2 = _pack(
        core, win, blk2, loc2, dstl, nwin, nchunk, zr2)

    # ---- x tables
    xb = x.astype(bf16)
    xblocks = []
    for q in range(nblk1):
        blkx = xb[q * L1_RANGE:q * L1_RANGE + blk1_rows[q]]
        xblocks.append(np.concatenate(
            [blkx, np.zeros((1, x.shape[1]), bf16)]))
    xdev = np.ascontiguousarray(np.concatenate(xblocks, axis=0))
    l1_base = np.concatenate(
        [[0], np.cumsum([b.shape[0] for b in xblocks])])[:-1]

    xts, dinvts = [], []
    for ci in range(NCORES):
        xs = x[ci * shard:(ci + 1) * shard]
        pad = nwin * P - shard
        xts.append(np.ascontiguousarray(np.concatenate(
            [xs, np.zeros((pad, x.shape[1]), np.float32)]).T).astype(bf16))
        dv = np.concatenate([deg_inv[ci * shard:(ci + 1) * shard],
                             np.zeros(pad, np.float32)])
        dinvts.append(np.ascontiguousarray(
            np.tile(dv[None, :], (P, 1))).astype(bf16))

    meta = dict(C01=C01, C1=C1, col1=col1, T1=T1, sumT1=sumT1,
                C02=C02, C2=C2, col2=col2, T2=T2, sumT2=sumT2,
                l1_base=l1_base, blk1_rows=blk1_rows)
    data = dict(xdev=xdev, idx1=idx1, dt1=dt1, idx2=idx2, dt2=dt2,
                xts=xts, dinvts=dinvts)
    return meta, data


# ---------------------------------------------------------------- builder

def _build(cfg, meta, repeat=1, debug=False, maxg=512, ablate=(),
           nq=4, single_packet=False, slab_bufs=3, batch_onehot=False,
           no_p2loc=False):
    import concourse.bacc as bacc
    import concourse.mybir as mybir
    import concourse.tile as tile

    f32 = mybir.dt.float32
    bf16 = mybir.dt.bfloat16
    i16 = mybir.dt.int16
    shard, nwin, ngrp = cfg["shard"], cfg["nwin"], cfg["ngrp"]
    nchunk, chunk_wins = cfg["nchunk"], cfg["chunk_wins"]
    chunk_rows, nblk1 = cfg["chunk_rows"], cfg["nblk1"]
    C01, C1, col1, T1, sumT1 = (meta[k] for k in
                                ("C01", "C1", "col1", "T1", "sumT1"))
    C02, C2, col2, T2, sumT2 = (meta[k] for k in
                                ("C02", "C2", "col2", "T2", "sumT2"))
    l1_base, blk1_rows = meta["l1_base"], meta["blk1_rows"]
    xdev_rows = int(l1_base[-1] + blk1_rows[-1] + 1)

    p2_off = np.concatenate(
        [[0], np.cumsum([NCORES * r + 1 for r in chunk_rows])])
    p2_rows = int(p2_off[-1])

    # group extents
    def grp_windows(g):
        return list(range(g * GRP, min((g + 1) * GRP, nwin)))

    def grp_cols(col, T, g, nblk):
        ws = grp_windows(g)
        a = int(col[ws[0], 0])
        last_b = nblk - 1
        e = int(col[ws[-1], last_b] + T[ws[-1], last_b])
        return a, e

    slab1_max = max(grp_cols(col1, T1, g, nblk1)[1] -
                    grp_cols(col1, T1, g, nblk1)[0] for g in range(ngrp))
    slab2_max = max(grp_cols(col2, T2, g, nchunk)[1] -
                    grp_cols(col2, T2, g, nchunk)[0] for g in range(ngrp))
    slab_max = max(slab1_max, slab2_max)
    s1w_max = int(C1.sum(axis=1).max())
    s2w_max = int(C2.sum(axis=1).max())
    sw_max = max(s1w_max, s2w_max)

    nc = bacc.Bacc(num_swdge_queues=nq)
    dp = nc.declare_dram_parameter
    xdev = dp("xdev", [xdev_rows, IN_CH], bf16, isOutput=False)
    xt = dp("xt", [P, nwin * P], bf16, isOutput=False)
    dinvt = dp("dinvt", [P, nwin * P], bf16, isOutput=False)
    idx1 = dp("idx1", [P, sumT1 // 16], i16, isOutput=False)
    dt1 = dp("dt1", [P, sumT1 // P], bf16, isOutput=False)
    idx2 = dp("idx2", [P, sumT2 // 16], i16, isOutput=False)
    dt2 = dp("dt2", [P, sumT2 // P], bf16, isOutput=False)
    w1l = dp("w1l", [IN_CH, HIDDEN], bf16, isOutput=False)
    w1r = dp("w1r", [IN_CH, HIDDEN], bf16, isOutput=False)
    w2lp = dp("w2lp", [HIDDEN, P], bf16, isOutput=False)   # zero-padded cols
    w2r = dp("w2r", [HIDDEN, OUT_CH], bf16, isOutput=False)
    b1c = dp("b1c", [P, 1], f32, isOutput=False)
    b2c = dp("b2c", [P, 1], f32, isOutput=False)
    iota = dp("iota", [P, P], bf16, isOutput=False)
    y = dp("y", [OUT_CH, nwin * P], bf16, isOutput=True)
    if debug:
        dbg_t1t = dp("dbg_t1t", [P, nwin * P], bf16, isOutput=True)
        dbg_ht = dp("dbg_ht", [P, nwin * P], bf16, isOutput=True)
        dbg_dinv = dp("dbg_dinv", [P, nwin * P], bf16, isOutput=True)
        dbg_raw = dp("dbg_raw", [P, nwin * P], f32, isOutput=True)
        dbg_gat = dp("dbg_gat", [P, 4096], bf16, isOutput=True)
        dbg_m = dp("dbg_m", [P, 4096], bf16, isOutput=True)

    p2_full = nc.dram_tensor("p2_full", [p2_rows, P], bf16,
                             addr_space="Shared")
    p2_loc = nc.dram_tensor("p2_loc", [p2_rows, P], bf16)

    with tile.TileContext(nc) as tc:
        with (
            tc.tile_pool(name="const", bufs=1) as cb,
            tc.tile_pool(name="slab", bufs=slab_bufs) as slb,
            tc.tile_pool(name="sb", bufs=3) as sb,
            tc.tile_pool(name="mt", bufs=2) as mtp,
            tc.tile_pool(name="ps", bufs=2, space="PSUM") as ps,
            tc.tile_pool(name="psb", bufs=2, space="PSUM") as psb,
            tc.tile_pool(name="dram", bufs=1, space="DRAM") as dr,
        ):
            def cload(param, shape, dtype, tag):
                t = cb.tile(shape, dtype, tag=tag)
                nc.sync.dma_start(out=t[:], in_=param[:])
                return t

            iota_t = cload(iota, [P, P], bf16, "c_iota")
            w1l_t = cload(w1l, [IN_CH, HIDDEN], bf16, "c_w1l")
            w1r_t = cload(w1r, [IN_CH, HIDDEN], bf16, "c_w1r")
            w2lp_t = cload(w2lp, [HIDDEN, P], bf16, "c_w2lp")
            w2r_t = cload(w2r, [HIDDEN, OUT_CH], bf16, "c_w2r")
            b1_t = cload(b1c, [P, 1], f32, "c_b1")
            b2_t = cload(b2c, [P, 1], f32, "c_b2")
            xt_t = cload(xt, [P, nwin * P], bf16, "c_xt")
            dinv_t = cload(dinvt, [P, nwin * P], bf16, "c_dinv")
            y_t = cb.tile([OUT_CH, nwin * P], bf16, tag="c_y")
            zrow_t = cb.tile([P, P], bf16, tag="c_zrow")
            nc.vector.memset(zrow_t[:], 0.0)

            p2c = []
            for c in range(nchunk):
                p2c_t = dr.tile([max(chunk_rows[c], 1), P], bf16,
                                tag=f"p2c{c}")
                p2c.append(p2c_t)
            for c in range(nchunk):
                zr = int(p2_off[c] + NCORES * chunk_rows[c])
                nc.sync.dma_start(out=p2_full[zr:zr + 1, :], in_=zrow_t[:1, :])
                if not no_p2loc:
                    nc.sync.dma_start(out=p2_loc[zr:zr + 1, :],
                                      in_=zrow_t[:1, :])
            p2_gsrc = p2_full if no_p2loc else p2_loc

            relu = mybir.ActivationFunctionType.Relu
            copyf = mybir.ActivationFunctionType.Copy
            iseq = mybir.AluOpType.is_equal
            mult = mybir.AluOpType.mult
            add = mybir.AluOpType.add

            for _rep in range(repeat):
                # ---------------- phase 1 ----------------
                for g in range(ngrp):
                    ws = grp_windows(g)
                    a, e = grp_cols(col1, T1, g, nblk1)
                    it = sb.tile([P, slab_max // 16], i16, tag="it")
                    nc.sync.dma_start(out=it[:, :(e - a) // 16],
                                      in_=idx1[:, a // 16:e // 16])
                    dtt = sb.tile([P, slab_max // P], bf16, tag="dtt")
                    nc.sync.dma_start(out=dtt[:, :(e - a) // P],
                                      in_=dt1[:, a // P:e // P])
                    gat = slb.tile([P, slab_max], bf16, tag="g")
                    for b in range(nblk1):
                        blo = int(l1_base[b])
                        nrows = blk1_rows[b] + 1
                        # split call ranges at cell boundaries, <= maxg idxs
                        spans = []
                        for w in ws:
                            t = int(T1[w, b])
                            if t == 0:
                                continue
                            c0 = int(col1[w, b])
                            if spans and spans[-1][1] - spans[-1][0] + t <= maxg:
                                spans[-1] = (spans[-1][0], c0 + t)
                            else:
                                spans.append((c0, c0 + t))
                        for si, (ca, ce) in enumerate(spans):
                            if "nogather" in ablate:
                                continue
                            t_q = ce - ca
                            nc.gpsimd.dma_gather(
                                out_ap=gat[:, ca - a:ce - a]
                                .rearrange("p (c e) -> p c e", e=IN_CH),
                                in_ap=xdev[blo:blo + nrows, :],
                                idxs_ap=it[:, (ca - a) // 16:(ce - a) // 16],
                                num_idxs=t_q,
                                num_idxs_reg=t_q,
                                elem_size=IN_CH,
                                single_packet=single_packet,
                                queue_num=(b + si) % nq,
                            )
                    if debug and g == 0:
                        nc.sync.dma_start(out=dbg_gat[:, :4096],
                                          in_=gat[:, :4096])
                    for w in ws:
                        s1w = int(C1[w].sum())
                        n_w = min(shard - w * P, P)
                        do_mm = s1w > 0 and "nomm" not in ablate
                        if do_mm:
                            # one-hot bank for all slices of this window
                            m_t = mtp.tile([P, sw_max * P], bf16, tag="m")
                            if "noonehot" not in ablate:
                                k = 0
                                for b in range(nblk1):
                                    c0 = int(C01[w, b]) - a // P
                                    cC = int(C1[w, b])
                                    if cC == 0:
                                        continue
                                    if batch_onehot:
                                        nc.vector.tensor_tensor(
                                            out=m_t[:, k * P:(k + cC) * P]
                                            .rearrange("p (c e) -> p c e",
                                                       e=P),
                                            in0=dtt[:, c0:c0 + cC]
                                            .unsqueeze(2)
                                            .broadcast_to([P, cC, P]),
                                            in1=iota_t[:, :P]
                                            .unsqueeze(1)
                                            .broadcast_to([P, cC, P]),
                                            op=iseq)
                                        k += cC
                                    else:
                                        for sß in range(cC):
                                            nc.vector.tensor_scalar(
                                                out=m_t[:, k * P:(k + 1) * P],
                                                in0=iota_t[:],
                                                scalar1=dtt[:, c0 + sß:
                                                            c0 + sß + 1],
                                                scalar2=None, op0=iseq)
                                            k += 1
                            psum1 = ps.tile([P, P], f32, tag="psA",
                                            space="PSUM")
                            k = 0
                            for b in range(nblk1):
                                c0 = int(C01[w, b])
                                for s in range(int(C1[w, b])):
                                    sl = (c0 + s) * P - a
                                    nc.tensor.matmul(
                                        out=psum1[:],
                                        lhsT=gat[:, sl:sl + P],
                                        rhs=m_t[:, k * P:(k + 1) * P],
                                        start=(k == 0), stop=(k == s1w - 1))
                                    k += 1
                            if debug and w == 0:
                                nc.sync.dma_start(
                                    out=dbg_m[:, :min(s1w * P, 4096)],
                                    in_=m_t[:, :min(s1w * P, 4096)])
                            t1t = sb.tile([P, P], bf16, tag="t1t")
                            nc.vector.tensor_tensor(
                                out=t1t[:], in0=psum1[:],
                                in1=dinv_t[:, w * P:(w + 1) * P], op=mult)
                        else:
                            t1t = sb.tile([P, P], bf16, tag="t1t")
                            nc.vector.memset(t1t[:], 0.0)
                        psum2 = psb.tile([P, P], f32, tag="ps2", space="PSUM")
                        nc.tensor.matmul(out=psum2[:], lhsT=w1l_t[:],
                                         rhs=t1t[:], start=True, stop=False)
                        nc.tensor.matmul(out=psum2[:], lhsT=w1r_t[:],
                                         rhs=xt_t[:, w * P:(w + 1) * P],
                                         start=False, stop=True)
                        ht = sb.tile([P, P], bf16, tag="ht")
                        nc.scalar.activation(out=ht[:], in_=psum2[:],
                                             func=relu, bias=b1_t[:, :1],
                                             scale=1.0)
                        if debug:
                            nc.sync.dma_start(
                                out=dbg_t1t[:, w * P:(w + 1) * P], in_=t1t[:])
                            nc.sync.dma_start(
                                out=dbg_ht[:, w * P:(w + 1) * P], in_=ht[:])
                            nc.sync.dma_start(
                                out=dbg_dinv[:, w * P:(w + 1) * P],
                                in_=dinv_t[:, w * P:(w + 1) * P])
                            if s1w > 0:
                                raw = sb.tile([P, P], f32, tag="dbgraw")
                                nc.vector.tensor_copy(out=raw[:], in_=psum1[:])
                                nc.sync.dma_start(
                                    out=dbg_raw[:, w * P:(w + 1) * P],
                                    in_=raw[:])
                        # p2 rows [n, 128] = ht^T @ W2l_pad
                        psum3 = psb.tile([P, P], f32, tag="ps3", space="PSUM")
                        nc.tensor.matmul(out=psum3[:], lhsT=ht[:],
                                         rhs=w2lp_t[:], start=True, stop=True)
                        p2sb = sb.tile([P, P], bf16, tag="p2sb")
                        nc.scalar.activation(out=p2sb[:], in_=psum3[:],
                                             func=copyf)
                        c = min(w // chunk_wins, nchunk - 1)
                        r0 = w * P - c * chunk_wins * P
                        nc.sync.dma_start(out=p2c[c][r0:r0 + n_w, :],
                                          in_=p2sb[:n_w, :])
                        if n_w < P:
                            nc.sync.dma_start(
                                out=p2c[c][r0 + n_w:r0 + P, :],
                                in_=zrow_t[:P - n_w, :])
                        # r2 = (h @ W2r)^T + b2 -> y accumulator
                        psum4 = psb.tile([OUT_CH, P], f32, tag="ps4",
                                         space="PSUM")
                        nc.tensor.matmul(out=psum4[:], lhsT=w2r_t[:],
                                         rhs=ht[:], start=True, stop=True)
                        nc.vector.tensor_scalar(
                            out=y_t[:, w * P:(w + 1) * P], in0=psum4[:],
                            scalar1=b2_t[:OUT_CH, :1], scalar2=None, op0=add)
                        if (w + 1) % chunk_wins == 0 or w == nwin - 1:
                            c_done = (min(w // chunk_wins, nchunk - 1))
                            if (w + 1) % chunk_wins == 0 or w == nwin - 1:
                                off = int(p2_off[c_done])
                                rows = NCORES * chunk_rows[c_done]
                                if "noag" in ablate:
                                    pass
                                elif NCORES > 1:
                                    nc.gpsimd.collective_compute(
                                        "AllGather",
                                        mybir.AluOpType.bypass,
                                        replica_groups=[list(range(NCORES))],
                                        ins=[p2c[c_done].opt()],
                                        outs=[p2_full[off:off + rows, :]],
                                    )
                                else:
                                    nc.sync.dma_start(
                                        out=p2_full[off:off + rows, :],
                                        in_=p2c[c_done][:, :])
                                if "noag" not in ablate and not no_p2loc:
                                    nc.sync.dma_start(
                                        out=p2_loc[off:off + rows, :],
                                        in_=p2_full[off:off + rows, :])

                # ---------------- phase 2 ----------------
                for g in range(ngrp if "nop2" not in ablate else 0):
                    ws = grp_windows(g)
                    a, e = grp_cols(col2, T2, g, nchunk)
                    it = sb.tile([P, slab_max // 16], i16, tag="it")
                    nc.sync.dma_start(out=it[:, :(e - a) // 16],
                                      in_=idx2[:, a // 16:e // 16])
                    dtt = sb.tile([P, slab_max // P], bf16, tag="dtt")
                    nc.sync.dma_start(out=dtt[:, :(e - a) // P],
                                      in_=dt2[:, a // P:e // P])
                    gat = slb.tile([P, slab_max], bf16, tag="g")
                    for b in range(nchunk):
                        off = int(p2_off[b])
                        nrows = NCORES * chunk_rows[b] + 1
                        spans = []
                        for w in ws:
                            t = int(T2[w, b])
                            if t == 0:
                                continue
                            c0 = int(col2[w, b])
                            if spans and spans[-1][1] - spans[-1][0] + t <= maxg:
                                spans[-1] = (spans[-1][0], c0 + t)
                            else:
                                spans.append((c0, c0 + t))
                        for si, (ca, ce) in enumerate(spans):
                            if "nogather" in ablate:
                                continue
                            t_q = ce - ca
                            nc.gpsimd.dma_gather(
                                out_ap=gat[:, ca - a:ce - a]
                                .rearrange("p (c e) -> p c e", e=P),
                                in_ap=p2_gsrc[off:off + nrows, :],
                                idxs_ap=it[:, (ca - a) // 16:(ce - a) // 16],
                                num_idxs=t_q,
                                num_idxs_reg=t_q,
                                elem_size=P,
                                single_packet=single_packet,
                                queue_num=(b + si) % nq,
                            )
                    for w in ws:
                        s2w = int(C2[w].sum())
                        if s2w == 0 or "nomm" in ablate:
                            continue
                        m_t = mtp.tile([P, sw_max * P], bf16, tag="m")
                        if "noonehot" not in ablate:
                            k = 0
                            for b in range(nchunk):
                                c0 = int(C02[w, b]) - a // P
                                cC = int(C2[w, b])
                                if cC == 0:
                                    continue
                                if batch_onehot:
                                    nc.vector.tensor_tensor(
                                        out=m_t[:, k * P:(k + cC) * P]
                                        .rearrange("p (c e) -> p c e", e=P),
                                        in0=dtt[:, c0:c0 + cC]
                                        .unsqueeze(2)
                                        .broadcast_to([P, cC, P]),
                                        in1=iota_t[:, :P]
                                        .unsqueeze(1)
                                        .broadcast_to([P, cC, P]),
                                        op=iseq)
                                    k += cC
                                else:
                                    for sß in range(cC):
                                        nc.vector.tensor_scalar(
                                            out=m_t[:, k * P:(k + 1) * P],
                                            in0=iota_t[:],
                                            scalar1=dtt[:, c0 + sß:c0 + sß + 1],
                                            scalar2=None, op0=iseq)
                                        k += 1
                        psum5f = ps.tile([P, P], f32, tag="psA",
                                         space="PSUM")
                        psum5 = psum5f[:OUT_CH, :]
                        k = 0
                        for b in range(nchunk):
                            c0 = int(C02[w, b])
                            for s in range(int(C2[w, b])):
                                sl = (c0 + s) * P - a
                                nc.tensor.matmul(
                                    out=psum5,
                                    lhsT=gat[:, sl:sl + OUT_CH],
                                    rhs=m_t[:, k * P:(k + 1) * P],
                                    start=(k == 0), stop=(k == s2w - 1))
                                k += 1
                        agg2 = sb.tile([OUT_CH, P], bf16, tag="agg2")
                        nc.vector.tensor_tensor(
                            out=agg2[:], in0=psum5,
                            in1=dinv_t[:OUT_CH, w * P:(w + 1) * P], op=mult)
                        nc.vector.tensor_tensor(
                            out=y_t[:, w * P:(w + 1) * P], in0=agg2[:],
                            in1=y_t[:, w * P:(w + 1) * P], op=add)

            nc.sync.dma_start(out=y[:, :], in_=y_t[:, :])

    nc.compile()
    return nc


# ---------------------------------------------------------------- v4 (RS)

GRP2 = 14                    # global windows per phase-2 group


def _pack_rs(core2, w2, sl, dpos, gw, zrow):
    """Pack src-sharded phase-2 edges into per-global-window cells (128-pad).

    Single gather block (the local p2 table). Edges sorted by local src
    within each cell for gather locality. Shapes shared across cores via
    max-over-cores cell sizes.
    """
    cnt = np.bincount(core2 * gw + w2,
                      minlength=NCORES * gw).reshape(NCORES, gw)
    T = (cnt.max(axis=0) + P - 1) // P * P
    col = np.concatenate([[0], np.cumsum(T)])[:-1]
    sumT = int(T.sum())
    W0 = col // P
    SL = T // P

    order = np.lexsort((sl, w2, core2))
    co, wo, so = core2[order], w2[order], sl[order]
    keys = co.astype(np.int64) * gw + wo
    newseg = np.ones(len(keys), bool)
    newseg[1:] = keys[1:] != keys[:-1]
    idxs = np.arange(len(keys))
    starts = idxs[newseg][np.cumsum(newseg) - 1]
    rank = idxs - starts
    pos = col[wo] + rank

    # Pads reference the cell's last real row per core (see _pack); a
    # shared zero row would hotspot one HBM bank across ~100k pad descs.
    fill = np.empty((NCORES, gw), np.int64)
    fill[:, :] = (np.arange(gw)[None, :] * 997) % zrow
    fill[co, wo] = so                      # last write wins per (core, w)
    idx_flat = np.empty((NCORES, sumT), np.int32)
    for w in range(gw):
        if T[w]:
            idx_flat[:, col[w]:col[w] + T[w]] = fill[:, w][:, None]
    dt_flat = np.full((NCORES, sumT), 999.0, np.float32)
    idx_flat[co, pos] = so
    dt_flat[co, pos] = dpos[order]
    idx16 = np.ascontiguousarray(
        idx_flat.reshape(NCORES, sumT // 16, 16).transpose(0, 2, 1)
    ).astype(np.int16)
    dt = np.ascontiguousarray(
        dt_flat.reshape(NCORES, sumT // P, P).transpose(0, 2, 1))
    return idx16, dt, W0, SL, col, T, sumT


def _preprocess_rs(x, edge_index, cfg):
    import ml_dtypes
    bf16 = ml_dtypes.bfloat16
    n = x.shape[0]
    shard, nwin = cfg["shard"], cfg["nwin"]
    nblk1 = cfg["nblk1"]
    gw = NCORES * nwin

    src = np.asarray(edge_index[0], dtype=np.int64)
    dst = np.asarray(edge_index[1], dtype=np.int64)
    deg = np.bincount(dst, minlength=n).astype(np.float32)
    deg_inv = np.where(deg > 0, np.float32(1.0) / np.maximum(deg, 1.0),
                       0.0).astype(np.float32)

    # ---- phase 1: dst-sharded edges, L1 cells over 4 src blocks
    core = (dst // shard).astype(np.int64)
    local = dst - core * shard
    win = local // P
    dstl = (local % P).astype(np.int32)
    blk1 = np.minimum(src // L1_RANGE, nblk1 - 1)
    loc1 = (src - blk1 * L1_RANGE).astype(np.int32)
    blk1_rows = [min(L1_RANGE, n - q * L1_RANGE) for q in range(nblk1)]
    idx1, dt1, C01, C1, col1, T1, sumT1 = _pack(
        core, win, blk1, loc1, dstl, nwin, nblk1, blk1_rows)

    # ---- phase 2: src-sharded edges, cells by padded-global dst window
    core2 = (src // shard).astype(np.int64)
    sl = (src - core2 * shard).astype(np.int32)
    gd = (nwin * P) * (dst // shard) + (dst % shard)
    w2 = (gd // P).astype(np.int64)
    dpos = (gd % P).astype(np.int32)
    idx2, dt2, W02, SL2, col2, T2, sumT2 = _pack_rs(
        core2, w2, sl, dpos, gw, shard)

    # ---- x tables
    xb = x.astype(bf16)
    xblocks = []
    for q in range(nblk1):
        blkx = xb[q * L1_RANGE:q * L1_RANGE + blk1_rows[q]]
        xblocks.append(np.concatenate(
            [blkx, np.zeros((1, x.shape[1]), bf16)]))
    xdev = np.ascontiguousarray(np.concatenate(xblocks, axis=0))
    l1_base = np.concatenate(
        [[0], np.cumsum([b.shape[0] for b in xblocks])])[:-1]

    xts, dinvts = [], []
    for ci in range(NCORES):
        xs = x[ci * shard:(ci + 1) * shard]
        pad = nwin * P - shard
        xts.append(np.ascontiguousarray(np.concatenate(
            [xs, np.zeros((pad, x.shape[1]), np.float32)]).T).astype(bf16))
        dv = np.concatenate([deg_inv[ci * shard:(ci + 1) * shard],
                             np.zeros(pad, np.float32)])
        dinvts.append(np.ascontiguousarray(
            np.tile(dv[None, :], (P, 1))).astype(bf16))

    meta = dict(C01=C01, C1=C1, col1=col1, T1=T1, sumT1=sumT1,
                W02=W02, SL2=SL2, col2=col2, T2=T2, sumT2=sumT2,
                l1_base=l1_base, blk1_rows=blk1_rows, gw=gw)
    data = dict(xdev=xdev, idx1=idx1, dt1=dt1, idx2=idx2, dt2=dt2,
                xts=xts, dinvts=dinvts)
    return meta, data


def _build_rs(cfg, meta, repeat=1, maxg=2048, nq=4, single_packet=False,
              slab_bufs=2, slab2_bufs=2, ablate=(), batch_onehot=True):
    import concourse.bacc as bacc
    import concourse.mybir as mybir
    import concourse.tile as tile

    f32 = mybir.dt.float32
    bf16 = mybir.dt.bfloat16
    i16 = mybir.dt.int16
    shard, nwin, ngrp = cfg["shard"], cfg["nwin"], cfg["ngrp"]
    nblk1 = cfg["nblk1"]
    gw = meta["gw"]
    C01, C1, col1, T1, sumT1 = (meta[k] for k in
                                ("C01", "C1", "col1", "T1", "sumT1"))
    W02, SL2, col2, T2, sumT2 = (meta[k] for k in
                                 ("W02", "SL2", "col2", "T2", "sumT2"))
    l1_base, blk1_rows = meta["l1_base"], meta["blk1_rows"]
    xdev_rows = int(l1_base[-1] + blk1_rows[-1] + 1)
    sh_pad = nwin * P
    ngrp2 = (gw + GRP2 - 1) // GRP2

    def grp_windows(g):
        return list(range(g * GRP, min((g + 1) * GRP, nwin)))

    def grp_cols1(g):
        ws = grp_windows(g)
        a = int(col1[ws[0], 0])
        e = int(col1[ws[-1], nblk1 - 1] + T1[ws[-1], nblk1 - 1])
        return a, e

    def grp2_windows(g):
        return list(range(g * GRP2, min((g + 1) * GRP2, gw)))

    def grp2_cols(g):
        ws = grp2_windows(g)
        return int(col2[ws[0]]), int(col2[ws[-1]] + T2[ws[-1]])

    slab1_max = max(grp_cols1(g)[1] - grp_cols1(g)[0] for g in range(ngrp))
    slab2_max = max(grp2_cols(g)[1] - grp2_cols(g)[0] for g in range(ngrp2))
    s1w_max = int(C1.sum(axis=1).max())
    s2w_max = int(SL2.max())

    nc = bacc.Bacc(num_swdge_queues=nq)
    dp = nc.declare_dram_parameter
    xdev = dp("xdev", [xdev_rows, IN_CH], bf16, isOutput=False)
    xt = dp("xt", [P, sh_pad], bf16, isOutput=False)
    dinvt = dp("dinvt", [P, sh_pad], bf16, isOutput=False)
    idx1 = dp("idx1", [P, sumT1 // 16], i16, isOutput=False)
    dt1 = dp("dt1", [P, sumT1 // P], bf16, isOutput=False)
    idx2 = dp("idx2", [P, sumT2 // 16], i16, isOutput=False)
    dt2 = dp("dt2", [P, sumT2 // P], bf16, isOutput=False)
    w1l = dp("w1l", [IN_CH, HIDDEN], bf16, isOutput=False)
    w1r = dp("w1r", [IN_CH, HIDDEN], bf16, isOutput=False)
    w2lp = dp("w2lp", [HIDDEN, P], bf16, isOutput=False)
    w2r = dp("w2r", [HIDDEN, OUT_CH], bf16, isOutput=False)
    b1c = dp("b1c", [P, 1], f32, isOutput=False)
    b2c = dp("b2c", [P, 1], f32, isOutput=False)
    iota = dp("iota", [P, P], bf16, isOutput=False)
    y = dp("y", [OUT_CH, sh_pad], bf16, isOutput=True)

    p2l = nc.dram_tensor("p2l", [shard + 1, P], bf16)
    # ch-major partials: core block cb occupies rows [cb*64, (cb+1)*64) —
    # flat-contiguous per block, so ReduceScatter hands core cb its own
    # [64, sh_pad] slice already in y_t orientation (no transposes).
    part = nc.dram_tensor("part", [NCORES * OUT_CH, sh_pad], bf16)
    rs_out = nc.dram_tensor("rs_out", [OUT_CH, sh_pad], bf16)
    grp_per_blk = nwin // GRP2
    assert nwin % GRP2 == 0, "phase-2 groups must not straddle core blocks"

    with tile.TileContext(nc) as tc:
        with (
            tc.tile_pool(name="const", bufs=1) as cb,
            tc.tile_pool(name="slab", bufs=slab_bufs) as slb,
            tc.tile_pool(name="slab2", bufs=slab2_bufs) as slb2,
            tc.tile_pool(name="sb", bufs=3) as sb,
            tc.tile_pool(name="mt", bufs=2) as mtp,
            tc.tile_pool(name="pg", bufs=2) as pgp,
            tc.tile_pool(name="ps", bufs=2, space="PSUM") as ps,
            tc.tile_pool(name="psb", bufs=2, space="PSUM") as psb,
        ):
            def cload(param, shape, dtype, tag):
                t = cb.tile(shape, dtype, tag=tag)
                nc.sync.dma_start(out=t[:], in_=param[:])
                return t

            iota_t = cload(iota, [P, P], bf16, "c_iota")
            w1l_t = cload(w1l, [IN_CH, HIDDEN], bf16, "c_w1l")
            w1r_t = cload(w1r, [IN_CH, HIDDEN], bf16, "c_w1r")
            w2lp_t = cload(w2lp, [HIDDEN, P], bf16, "c_w2lp")
            w2r_t = cload(w2r, [HIDDEN, OUT_CH], bf16, "c_w2r")
            b1_t = cload(b1c, [P, 1], f32, "c_b1")
            b2_t = cload(b2c, [P, 1], f32, "c_b2")
            xt_t = cload(xt, [P, sh_pad], bf16, "c_xt")
            dinv_t = cload(dinvt, [P, sh_pad], bf16, "c_dinv")
            y_t = cb.tile([OUT_CH, sh_pad], bf16, tag="c_y")
            zrow_t = cb.tile([P, P], bf16, tag="c_zrow")
            nc.vector.memset(zrow_t[:], 0.0)
            nc.sync.dma_start(out=p2l[shard:shard + 1, :], in_=zrow_t[:1, :])

            relu = mybir.ActivationFunctionType.Relu
            copyf = mybir.ActivationFunctionType.Copy
            iseq = mybir.AluOpType.is_equal
            mult = mybir.AluOpType.mult
            add = mybir.AluOpType.add

            for _rep in range(repeat):
                # ---------------- phase 1 (dst-sharded, as v2) ----------
                for g in range(ngrp):
                    ws = grp_windows(g)
                    a, e = grp_cols1(g)
                    it = sb.tile([P, slab1_max // 16], i16, tag="it")
                    nc.sync.dma_start(out=it[:, :(e - a) // 16],
                                      in_=idx1[:, a // 16:e // 16])
                    dtt = sb.tile([P, slab1_max // P], bf16, tag="dtt")
                    nc.sync.dma_start(out=dtt[:, :(e - a) // P],
                                      in_=dt1[:, a // P:e // P])
                    gat = slb.tile([P, slab1_max], bf16, tag="g")
                    for b in range(nblk1):
                        blo = int(l1_base[b])
                        nrows = blk1_rows[b] + 1
                        spans = []
                        for w in ws:
                            t = int(T1[w, b])
                            if t == 0:
                                continue
                            c0 = int(col1[w, b])
                            if spans and spans[-1][1] - spans[-1][0] + t <= maxg:
                                spans[-1] = (spans[-1][0], c0 + t)
                            else:
                                spans.append((c0, c0 + t))
                        for si, (ca, ce) in enumerate(spans):
                            if "nogather" in ablate:
                                continue
                            t_q = ce - ca
                            nc.gpsimd.dma_gather(
                                out_ap=gat[:, ca - a:ce - a]
                                .rearrange("p (c e) -> p c e", e=IN_CH),
                                in_ap=xdev[blo:blo + nrows, :],
                                idxs_ap=it[:, (ca - a) // 16:(ce - a) // 16],
                                num_idxs=t_q,
                                num_idxs_reg=t_q,
                                elem_size=IN_CH,
                                single_packet=single_packet,
                                queue_num=(b + si) % nq,
                            )
                    for w in ws:
                        s1w = int(C1[w].sum())
                        n_w = min(shard - w * P, P)
                        if s1w > 0:
                            m_t = mtp.tile([P, s1w_max * P], bf16, tag="m")
                            k = 0
                            for b in range(nblk1):
                                c0 = int(C01[w, b]) - a // P
                                cC = int(C1[w, b])
                                if cC == 0:
                                    continue
                                if batch_onehot:
                                    nc.vector.tensor_tensor(
                                        out=m_t[:, k * P:(k + cC) * P]
                                        .rearrange("p (c e) -> p c e", e=P),
                                        in0=dtt[:, c0:c0 + cC]
                                        .unsqueeze(2)
                                        .broadcast_to([P, cC, P]),
                                        in1=iota_t[:, :P]
                                        .unsqueeze(1)
                                        .broadcast_to([P, cC, P]),
                                        op=iseq)
                                    k += cC
                                else:
                                    for s in range(cC):
                                        nc.vector.tensor_scalar(
                                            out=m_t[:, k * P:(k + 1) * P],
                                            in0=iota_t[:],
                                            scalar1=dtt[:, c0 + s:c0 + s + 1],
                                            scalar2=None, op0=iseq)
                                        k += 1
                            psum1 = ps.tile([P, P], f32, tag="psA",
                                            space="PSUM")
                            k = 0
                            for b in range(nblk1):
                                c0 = int(C01[w, b])
                                for s in range(int(C1[w, b])):
                                    sl_ = (c0 + s) * P - a
                                    nc.tensor.matmul(
                                        out=psum1[:],
                                        lhsT=gat[:, sl_:sl_ + P],
                                        rhs=m_t[:, k * P:(k + 1) * P],
                                        start=(k == 0), stop=(k == s1w - 1))
                                    k += 1
                            t1t = sb.tile([P, P], bf16, tag="t1t")
                            nc.vector.tensor_tensor(
                                out=t1t[:], in0=psum1[:],
                                in1=dinv_t[:, w * P:(w + 1) * P], op=mult)
                        else:
                            t1t = sb.tile([P, P], bf16, tag="t1t")
                            nc.vector.memset(t1t[:], 0.0)
                        psum2 = psb.tile([P, P], f32, tag="ps2", space="PSUM")
                        nc.tensor.matmul(out=psum2[:], lhsT=w1l_t[:],
                                         rhs=t1t[:], start=True, stop=False)
                        nc.tensor.matmul(out=psum2[:], lhsT=w1r_t[:],
                                         rhs=xt_t[:, w * P:(w + 1) * P],
                                         start=False, stop=True)
                        ht = sb.tile([P, P], bf16, tag="ht")
                        nc.scalar.activation(out=ht[:], in_=psum2[:],
                                             func=relu, bias=b1_t[:, :1],
                                             scale=1.0)
                        # p2 rows -> local table
                        psum3 = psb.tile([P, P], f32, tag="ps3", space="PSUM")
                        nc.tensor.matmul(out=psum3[:], lhsT=ht[:],
                                         rhs=w2lp_t[:], start=True, stop=True)
                        p2sb = sb.tile([P, P], bf16, tag="p2sb")
                        nc.scalar.activation(out=p2sb[:], in_=psum3[:],
                                             func=copyf)
                        nc.sync.dma_start(out=p2l[w * P:w * P + n_w, :],
                                          in_=p2sb[:n_w, :])
                        # root term -> y accumulator
                        psum4 = psb.tile([OUT_CH, P], f32, tag="ps4",
                                         space="PSUM")
                        nc.tensor.matmul(out=psum4[:], lhsT=w2r_t[:],
                                         rhs=ht[:], start=True, stop=True)
                        nc.vector.tensor_scalar(
                            out=y_t[:, w * P:(w + 1) * P], in0=psum4[:],
                            scalar1=b2_t[:OUT_CH, :1], scalar2=None, op0=add)

                # ---------------- phase 2 (src-sharded partials) --------
                for g in range(ngrp2 if "nop2" not in ablate else 0):
                    ws2 = grp2_windows(g)
                    a, e = grp2_cols(g)
                    it = sb.tile([P, slab2_max // 16], i16, tag="it2")
                    nc.sync.dma_start(out=it[:, :(e - a) // 16],
                                      in_=idx2[:, a // 16:e // 16])
                    dtt = sb.tile([P, slab2_max // P], bf16, tag="dtt2")
                    nc.sync.dma_start(out=dtt[:, :(e - a) // P],
                                      in_=dt2[:, a // P:e // P])
                    gat = slb2.tile([P, slab2_max], bf16, tag="g2")
                    spans = []
                    for w in ws2:
                        t = int(T2[w])
                        if t == 0:
                            continue
                        c0 = int(col2[w])
                        if spans and spans[-1][1] - spans[-1][0] + t <= maxg:
                            spans[-1] = (spans[-1][0], c0 + t)
                        else:
                            spans.append((c0, c0 + t))
                    for si, (ca, ce) in enumerate(spans):
                        if "nogather" in ablate:
                            continue
                        t_q = ce - ca
                        nc.gpsimd.dma_gather(
                            out_ap=gat[:, ca - a:ce - a]
                            .rearrange("p (c e) -> p c e", e=P),
                            in_ap=p2l[:, :],
                            idxs_ap=it[:, (ca - a) // 16:(ce - a) // 16],
                            num_idxs=t_q,
                            num_idxs_reg=t_q,
                            elem_size=P,
                            single_packet=single_packet,
                            queue_num=si % nq,
                        )
                    pg = pgp.tile([OUT_CH, GRP2 * P], bf16, tag="pg")
                    for wi, w in enumerate(ws2):
                        s2w = int(SL2[w])
                        if s2w == 0:
                            nc.vector.memset(
                                pg[:, wi * P:(wi + 1) * P], 0.0)
                            continue
                        m_t = mtp.tile([P, s2w_max * P], bf16, tag="m2")
                        c0 = int(W02[w]) - a // P
                        nc.vector.tensor_tensor(
                            out=m_t[:, :s2w * P]
                            .rearrange("p (c e) -> p c e", e=P),
                            in0=dtt[:, c0:c0 + s2w]
                            .unsqueeze(2).broadcast_to([P, s2w, P]),
                            in1=iota_t[:, :P]
                            .unsqueeze(1).broadcast_to([P, s2w, P]),
                            op=iseq)
                        psum5f = ps.tile([P, P], f32, tag="psA",
                                         space="PSUM")
                        psum5 = psum5f[:OUT_CH, :]
                        for s in range(s2w):
                            sl_ = (int(W02[w]) + s) * P - a
                            nc.tensor.matmul(
                                out=psum5,
                                lhsT=gat[:, sl_:sl_ + OUT_CH],
                                rhs=m_t[:, s * P:(s + 1) * P],
                                start=(s == 0), stop=(s == s2w - 1))
                        nc.scalar.activation(
                            out=pg[:, wi * P:(wi + 1) * P],
                            in_=psum5, func=copyf)
                    if "nopart" not in ablate:
                        cb = g // grp_per_blk
                        lw0 = (g % grp_per_blk) * GRP2
                        nc.sync.dma_start(
                            out=part[cb * OUT_CH:(cb + 1) * OUT_CH,
                                     lw0 * P:lw0 * P + len(ws2) * P],
                            in_=pg[:, :len(ws2) * P])

                # ---------------- ReduceScatter + combine ---------------
                if "nop2" not in ablate and "nocomb" not in ablate:
                    if "nors" not in ablate:
                        nc.gpsimd.collective_compute(
                            "ReduceScatter",
                            mybir.AluOpType.add,
                            replica_groups=[list(range(NCORES))],
                            ins=[part[:, :]],
                            outs=[rs_out[:, :]],
                        )
                    for g in range(ngrp):
                        ws = grp_windows(g)
                        r0 = ws[0] * P
                        nr = len(ws) * P
                        rsc = sb.tile([OUT_CH, GRP * P], bf16, tag="rsc")
                        nc.sync.dma_start(out=rsc[:, :nr],
                                          in_=rs_out[:, r0:r0 + nr])
                        agg = sb.tile([OUT_CH, GRP * P], bf16, tag="aggg")
                        nc.vector.tensor_tensor(
                            out=agg[:, :nr], in0=rsc[:, :nr],
                            in1=dinv_t[:OUT_CH, r0:r0 + nr], op=mult)
                        nc.vector.tensor_tensor(
                            out=y_t[:, r0:r0 + nr],
                            in0=agg[:, :nr],
                            in1=y_t[:, r0:r0 + nr], op=add)

            nc.sync.dma_start(out=y[:, :], in_=y_t[:, :])

    nc.compile()
    return nc


def _make_inmaps_rs(inputs, meta, data):
    import ml_dtypes
    bf16 = ml_dtypes.bfloat16
    iota_v = np.tile(np.arange(P, dtype=np.float32), (P, 1)).astype(bf16)
    id_v = np.eye(P, dtype=np.float32)
    w2l = np.asarray(inputs["W2_l"], np.float32)
    w2lp = np.concatenate(
        [w2l, np.zeros((HIDDEN, P - OUT_CH), np.float32)], axis=1)
    b2 = np.asarray(inputs["b2"], np.float32)
    common = dict(
        xdev=data["xdev"],
        w1l=np.asarray(inputs["W1_l"], np.float32).astype(bf16),
        w1r=np.asarray(inputs["W1_r"], np.float32).astype(bf16),
        w2lp=w2lp.astype(bf16),
        w2r=np.asarray(inputs["W2_r"], np.float32).astype(bf16),
        b1c=np.asarray(inputs["b1"], np.float32).reshape(P, 1),
        b2c=np.concatenate([b2, np.zeros(P - OUT_CH, np.float32)]
                           ).reshape(P, 1),
        iota=iota_v,
        id128=id_v.astype(bf16),
    )
    in_maps = []
    for ci in range(NCORES):
        m = dict(common)
        m["xt"] = data["xts"][ci]
        m["dinvt"] = data["dinvts"][ci]
        m["idx1"] = np.tile(data["idx1"][ci], (8, 1))
        m["dt1"] = data["dt1"][ci].astype(bf16)
        m["idx2"] = np.tile(data["idx2"][ci], (8, 1))
        m["dt2"] = data["dt2"][ci].astype(bf16)
        in_maps.append(m)
    return in_maps


# ---------------------------------------------------------------- entry

_CACHE = {}


def _meta_sig(meta):
    return (int(meta["sumT1"]), int(meta["sumT2"]))


def _make_inmaps(inputs, meta, data):
    import ml_dtypes
    bf16 = ml_dtypes.bfloat16
    iota_v = np.tile(np.arange(P, dtype=np.float32),
                     (P, 1)).astype(bf16)
    w2l = np.asarray(inputs["W2_l"], np.float32)
    w2lp = np.concatenate(
        [w2l, np.zeros((HIDDEN, P - OUT_CH), np.float32)], axis=1)
    b2 = np.asarray(inputs["b2"], np.float32)
    common = dict(
        xdev=data["xdev"],
        w1l=np.asarray(inputs["W1_l"], np.float32).astype(bf16),
        w1r=np.asarray(inputs["W1_r"], np.float32).astype(bf16),
        w2lp=w2lp.astype(bf16),
        w2r=np.asarray(inputs["W2_r"], np.float32).astype(bf16),
        b1c=np.asarray(inputs["b1"], np.float32).reshape(P, 1),
        b2c=np.concatenate([b2, np.zeros(P - OUT_CH, np.float32)]
                           ).reshape(P, 1),
        iota=iota_v,
    )
    in_maps = []
    for ci in range(NCORES):
        m = dict(common)
        m["xt"] = data["xts"][ci]
        m["dinvt"] = data["dinvts"][ci]
        m["idx1"] = np.tile(data["idx1"][ci], (8, 1))
        m["dt1"] = data["dt1"][ci].astype(bf16)
        m["idx2"] = np.tile(data["idx2"][ci], (8, 1))
        m["dt2"] = data["dt2"][ci].astype(bf16)
        in_maps.append(m)
    return in_maps


# Which kernel strategy kernel() uses: "rs" (src-sharded phase 2 +
# ReduceScatter, v4) or "ag" (dst-sharded + AllGather, v2/v3lite).
# HW-measured: ag/maxg2048 ~1.28 ms/iter vs rs best ~1.32 ms/iter.
KERNEL_KIND = "ag"
BUILD_KW = dict(maxg=1024)
BUILD_KW_AG = dict(maxg=2048, no_p2loc=True, batch_onehot=True)


def _prep_any(x, edge_index, cfg):
    if KERNEL_KIND == "rs":
        return _preprocess_rs(x, edge_index, cfg)
    return _preprocess(x, edge_index, cfg)


def _build_any(cfg, meta, repeat=1, **kw):
    if KERNEL_KIND == "rs":
        return _build_rs(cfg, meta, repeat=repeat, **{**BUILD_KW, **kw})
    return _build(cfg, meta, repeat=repeat, **{**BUILD_KW_AG, **kw})


def _inmaps_any(inputs, meta, data):
    if KERNEL_KIND == "rs":
        return _make_inmaps_rs(inputs, meta, data)
    return _make_inmaps(inputs, meta, data)


def kernel(x, edge_index, W1_l, W1_r, b1, W2_l, W2_r, b2):
    x = np.asarray(x, dtype=np.float32)
    edge_index = np.asarray(edge_index)
    cfg = _derive_cfg(x.shape[0])
    meta, data = _prep_any(x, edge_index, cfg)

    key = (KERNEL_KIND, x.shape, edge_index.shape)
    if key in _CACHE and _CACHE[key][1] == _meta_sig(meta):
        nc = _CACHE[key][0]
    else:
        nc = _build_any(cfg, meta)
        _CACHE[key] = (nc, _meta_sig(meta))

    in_maps = _inmaps_any(
        dict(W1_l=W1_l, W1_r=W1_r, b1=b1, W2_l=W2_l, W2_r=W2_r, b2=b2),
        meta, data)

    from concourse.bass_utils import run_bass_kernel_spmd
    r = run_bass_kernel_spmd(nc, in_maps, core_ids=list(range(NCORES)))
    shard = cfg["shard"]
    out = np.concatenate(
        [r.results[c]["y"].astype(np.float32).T[:shard]
         for c in range(NCORES)], axis=0)
    return np.ascontiguousarray(out, dtype=np.float32)



# revision 35
# speedup vs baseline: 1.6483x; 1.0418x over previous
"""Two-layer GraphSAGE (mean aggregation) on 8 Trainium2 NeuronCores.

Default strategy ("ag"): dst-sharded nodes/edges, replicated weights,
AllGather for the layer-2 halo exchange, tuned for the memory roofline:

  * All compute in bf16 (PSUM accumulation stays fp32); dst-position and
    iota tables in bf16 too (2x DVE throughput for the one-hot builds).
  * Edges are packed on the host into (window, src-block) cells padded to
    128 so every gathered slab slice belongs to exactly one 128-node dst
    window. Gather calls merge adjacent cells up to maxg=3072 indices
    (SWDGE desc-gen has ~1us fixed cost per call).
  * Cell-pad gather slots reference the cell's last real row, NOT a shared
    zero row: their values are killed by the all-zero one-hot column, and
    a shared row would serialize ~100k pad descriptors on one HBM bank
    (measured ~2.5 ms penalty before this fix).
  * Per-cell one-hot segment matrices are built by one DVE
    tensor_tensor(is_equal) over stride-0 broadcast access patterns.
  * deg_inv is folded in via a host-tiled [128, nwin*128] bf16 table:
    mean^T = psum * dinv_tile (one DVE op, PSUM->SBUF, no transposes).
  * Layer 2 transforms first (p2 = h @ W2_l, padded to 128 cols so the
    bf16 gather element is 256B), AllGathers p2 in 4 chunks overlapped
    with phase 1, then gathers p2 rows directly from the Shared AllGather
    output (no local copy).
  * Output y accumulates in a resident [64, nwin*128] bf16 tile; one DMA.

An alternative "rs" strategy (src-sharded layer 2: gather from the core's
own 3.2 MB p2 table, ch-major partial tables, one ReduceScatter) is kept
in _preprocess_rs/_build_rs; it measured ~3% slower on HW.

Self-contained: hardcodes problem shapes from the task spec.
"""

import numpy as np

IN_CH, HIDDEN, OUT_CH = 128, 128, 64
N_NODES, N_EDGES = 100000, 1600000
NCORES = 8
P = 128
L1_RANGE = 25000             # src rows per L1 gather block (int16 idx limit)
GRP = 7                      # windows per gather group


def _derive_cfg(n_nodes, nchunk=None):
    shard = n_nodes // NCORES
    nwin = (shard + P - 1) // P
    ngrp = (nwin + GRP - 1) // GRP
    if nchunk is None:
        nchunk = 4 if nwin >= 4 else 1
    chunk_wins = (nwin + nchunk - 1) // nchunk
    chunk_rows = []
    for c in range(nchunk):
        lo = c * chunk_wins * P
        hi = min((c + 1) * chunk_wins * P, nwin * P)
        chunk_rows.append(max(hi - lo, 0))
    nblk1 = (n_nodes + L1_RANGE - 1) // L1_RANGE
    return dict(shard=shard, nwin=nwin, ngrp=ngrp, nchunk=nchunk,
                chunk_wins=chunk_wins, chunk_rows=chunk_rows, nblk1=nblk1)


def _pack(core, win, blk, loc, dstl, nwin, nblk, zero_rows,
          sort_loc=False):
    """Pack edges into per-(window, block) cells padded to 128.

    Cell order: (group, block, window-in-group)  [gather-call layout].
    sort_loc orders edges by source row within each cell so gather
    descriptors walk ascending HBM addresses.
    Returns idx16 [NCORES, 16, sumT//16], dt [NCORES, 128, sumC] (999 pad),
    cell slice-offset table C0 [nwin, nblk], cell slice counts C [nwin, nblk].
    """
    ngrp = (nwin + GRP - 1) // GRP
    cnt = np.bincount((core * nwin + win) * nblk + blk,
                      minlength=NCORES * nwin * nblk
                      ).reshape(NCORES, nwin, nblk)
    T = cnt.max(axis=0)                       # [nwin, nblk]
    T = (T + P - 1) // P * P                  # pad cells to 128
    C = T // P                                # slices per cell

    # global cell order (g, b, w_in_g)
    order_cells = []
    for g in range(ngrp):
        ws = range(g * GRP, min((g + 1) * GRP, nwin))
        for b in range(nblk):
            for w in ws:
                order_cells.append((w, b))
    cell_rank = np.full((nwin, nblk), -1, np.int64)
    col = np.zeros((nwin, nblk), np.int64)    # start column (edge units)
    acc = 0
    for r, (w, b) in enumerate(order_cells):
        cell_rank[w, b] = r
        col[w, b] = acc
        acc += T[w, b]
    sumT = acc
    C0 = col // P                             # start slice index

    # per-edge destination position
    cellid = cell_rank[win, blk]              # [E]
    keys = core.astype(np.int64) * len(order_cells) + cellid
    if sort_loc:
        order = np.lexsort((loc, keys))
    else:
        order = np.argsort(keys, kind="stable")
    ks = keys[order]
    # rank within (core, cell)
    first = np.zeros(len(ks), np.int64)
    if len(ks):
        newseg = np.ones(len(ks), bool)
        newseg[1:] = ks[1:] != ks[:-1]
        seg_starts = np.flatnonzero(newseg)
        first[seg_starts] = 1
        idxs = np.arange(len(ks))
        starts_for = idxs[newseg][np.cumsum(newseg) - 1]
        rank = idxs - starts_for
    else:
        rank = first
    pos = col[win[order], blk[order]] + rank  # col within core's table

    # Pad slots must reference SOME valid row (their one-hot column is
    # all-zero, so the value is never used). Use the cell's last real row
    # per core (row-buffer hit on HBM) — a shared zero row would turn the
    # pad descriptors into a single-bank hotspot.
    co = core[order]
    fill = np.empty((NCORES, nwin, nblk), np.int64)
    for b in range(nblk):
        fill[:, :, b] = (np.arange(nwin)[None, :] * 997) % zero_rows[b]
    fill[co, win[order], blk[order]] = loc[order]   # last write wins
    idx_flat = np.empty((NCORES, sumT), np.int32)
    for b in range(nblk):
        for w in range(nwin):
            if T[w, b]:
                idx_flat[:, col[w, b]:col[w, b] + T[w, b]] = \
                    fill[:, w, b][:, None]
    dt_flat = np.full((NCORES, sumT), 999.0, np.float32)
    idx_flat[co, pos] = loc[order]
    dt_flat[co, pos] = dstl[order]

    # wrap idx into 16 partitions per gather call (call = (g, b) range)
    idx16 = np.empty((NCORES, 16, sumT // 16), np.int16)
    for g in range(ngrp):
        ws = list(range(g * GRP, min((g + 1) * GRP, nwin)))
        for b in range(nblk):
            a = int(col[ws[0], b])
            e = int(col[ws[-1], b] + T[ws[-1], b])
            if e == a:
                continue
            seg = idx_flat[:, a:e]
            idx16[:, :, a // 16:e // 16] = (
                seg.reshape(NCORES, (e - a) // 16, 16).transpose(0, 2, 1))
    dt = dt_flat.reshape(NCORES, sumT // P, P).transpose(0, 2, 1)
    return idx16, np.ascontiguousarray(dt), C0, C, col, T, sumT


def _preprocess(x, edge_index, cfg, sort_loc=False):
    import ml_dtypes
    bf16 = ml_dtypes.bfloat16
    n = x.shape[0]
    shard, nwin = cfg["shard"], cfg["nwin"]
    nchunk, chunk_wins = cfg["nchunk"], cfg["chunk_wins"]
    chunk_rows, nblk1 = cfg["chunk_rows"], cfg["nblk1"]

    src = np.asarray(edge_index[0], dtype=np.int64)
    dst = np.asarray(edge_index[1], dtype=np.int64)
    deg = np.bincount(dst, minlength=n).astype(np.float32)
    deg_inv = np.where(deg > 0, np.float32(1.0) / np.maximum(deg, 1.0),
                       0.0).astype(np.float32)

    core = (dst // shard).astype(np.int64)
    local = dst - core * shard
    win = local // P
    dstl = (local % P).astype(np.int32)

    # ---- L1 cells
    blk1 = np.minimum(src // L1_RANGE, nblk1 - 1)
    loc1 = (src - blk1 * L1_RANGE).astype(np.int32)
    blk1_rows = [min(L1_RANGE, n - q * L1_RANGE) for q in range(nblk1)]
    zr1 = blk1_rows                    # zero row index per block
    idx1, dt1, C01, C1, col1, T1, sumT1 = _pack(
        core, win, blk1, loc1, dstl, nwin, nblk1, zr1, sort_loc=sort_loc)

    # ---- L2 cells (blocks = AllGather chunks)
    csz = chunk_wins * P
    blk2 = np.minimum((src % shard) // csz, nchunk - 1)
    cr = np.array(chunk_rows)
    loc2 = ((src // shard) * cr[blk2] + (src % shard) - blk2 * csz
            ).astype(np.int32)
    zr2 = [NCORES * r for r in chunk_rows]
    idx2, dt2, C02, C2, col2, T2, sumT2 = _pack(
        core, win, blk2, loc2, dstl, nwin, nchunk, zr2, sort_loc=sort_loc)

    # ---- x tables
    xb = x.astype(bf16)
    xblocks = []
    for q in range(nblk1):
        blkx = xb[q * L1_RANGE:q * L1_RANGE + blk1_rows[q]]
        xblocks.append(np.concatenate(
            [blkx, np.zeros((1, x.shape[1]), bf16)]))
    xdev = np.ascontiguousarray(np.concatenate(xblocks, axis=0))
    l1_base = np.concatenate(
        [[0], np.cumsum([b.shape[0] for b in xblocks])])[:-1]

    xts, dinvts = [], []
    for ci in range(NCORES):
        xs = x[ci * shard:(ci + 1) * shard]
        pad = nwin * P - shard
        xts.append(np.ascontiguousarray(np.concatenate(
            [xs, np.zeros((pad, x.shape[1]), np.float32)]).T).astype(bf16))
        dv = np.concatenate([deg_inv[ci * shard:(ci + 1) * shard],
                             np.zeros(pad, np.float32)])
        dinvts.append(np.ascontiguousarray(
            np.tile(dv[None, :], (P, 1))).astype(bf16))

    meta = dict(C01=C01, C1=C1, col1=col1, T1=T1, sumT1=sumT1,
                C02=C02, C2=C2, col2=col2, T2=T2, sumT2=sumT2,
                l1_base=l1_base, blk1_rows=blk1_rows)
    data = dict(xdev=xdev, idx1=idx1, dt1=dt1, idx2=idx2, dt2=dt2,
                xts=xts, dinvts=dinvts)
    return meta, data


# ---------------------------------------------------------------- builder

def _build(cfg, meta, repeat=1, debug=False, maxg=512, ablate=(),
           nq=4, single_packet=False, slab_bufs=3, batch_onehot=False,
           no_p2loc=False, q_by_block=False):
    import concourse.bacc as bacc
    import concourse.mybir as mybir
    import concourse.tile as tile

    f32 = mybir.dt.float32
    bf16 = mybir.dt.bfloat16
    i16 = mybir.dt.int16
    shard, nwin, ngrp = cfg["shard"], cfg["nwin"], cfg["ngrp"]
    nchunk, chunk_wins = cfg["nchunk"], cfg["chunk_wins"]
    chunk_rows, nblk1 = cfg["chunk_rows"], cfg["nblk1"]
    C01, C1, col1, T1, sumT1 = (meta[k] for k in
                                ("C01", "C1", "col1", "T1", "sumT1"))
    C02, C2, col2, T2, sumT2 = (meta[k] for k in
                                ("C02", "C2", "col2", "T2", "sumT2"))
    l1_base, blk1_rows = meta["l1_base"], meta["blk1_rows"]
    xdev_rows = int(l1_base[-1] + blk1_rows[-1] + 1)

    p2_off = np.concatenate(
        [[0], np.cumsum([NCORES * r + 1 for r in chunk_rows])])
    p2_rows = int(p2_off[-1])

    # group extents
    def grp_windows(g):
        return list(range(g * GRP, min((g + 1) * GRP, nwin)))

    def grp_cols(col, T, g, nblk):
        ws = grp_windows(g)
        a = int(col[ws[0], 0])
        last_b = nblk - 1
        e = int(col[ws[-1], last_b] + T[ws[-1], last_b])
        return a, e

    slab1_max = max(grp_cols(col1, T1, g, nblk1)[1] -
                    grp_cols(col1, T1, g, nblk1)[0] for g in range(ngrp))
    slab2_max = max(grp_cols(col2, T2, g, nchunk)[1] -
                    grp_cols(col2, T2, g, nchunk)[0] for g in range(ngrp))
    slab_max = max(slab1_max, slab2_max)
    s1w_max = int(C1.sum(axis=1).max())
    s2w_max = int(C2.sum(axis=1).max())
    sw_max = max(s1w_max, s2w_max)

    nc = bacc.Bacc(num_swdge_queues=nq)
    dp = nc.declare_dram_parameter
    xdev = dp("xdev", [xdev_rows, IN_CH], bf16, isOutput=False)
    xt = dp("xt", [P, nwin * P], bf16, isOutput=False)
    dinvt = dp("dinvt", [P, nwin * P], bf16, isOutput=False)
    idx1 = dp("idx1", [P, sumT1 // 16], i16, isOutput=False)
    dt1 = dp("dt1", [P, sumT1 // P], bf16, isOutput=False)
    idx2 = dp("idx2", [P, sumT2 // 16], i16, isOutput=False)
    dt2 = dp("dt2", [P, sumT2 // P], bf16, isOutput=False)
    w1l = dp("w1l", [IN_CH, HIDDEN], bf16, isOutput=False)
    w1r = dp("w1r", [IN_CH, HIDDEN], bf16, isOutput=False)
    w2lp = dp("w2lp", [HIDDEN, P], bf16, isOutput=False)   # zero-padded cols
    w2r = dp("w2r", [HIDDEN, OUT_CH], bf16, isOutput=False)
    b1c = dp("b1c", [P, 1], f32, isOutput=False)
    b2c = dp("b2c", [P, 1], f32, isOutput=False)
    iota = dp("iota", [P, P], bf16, isOutput=False)
    y = dp("y", [OUT_CH, nwin * P], bf16, isOutput=True)
    if debug:
        dbg_t1t = dp("dbg_t1t", [P, nwin * P], bf16, isOutput=True)
        dbg_ht = dp("dbg_ht", [P, nwin * P], bf16, isOutput=True)
        dbg_dinv = dp("dbg_dinv", [P, nwin * P], bf16, isOutput=True)
        dbg_raw = dp("dbg_raw", [P, nwin * P], f32, isOutput=True)
        dbg_gat = dp("dbg_gat", [P, 4096], bf16, isOutput=True)
        dbg_m = dp("dbg_m", [P, 4096], bf16, isOutput=True)

    p2_full = nc.dram_tensor("p2_full", [p2_rows, P], bf16,
                             addr_space="Shared")
    p2_loc = nc.dram_tensor("p2_loc", [p2_rows, P], bf16)

    with tile.TileContext(nc) as tc:
        with (
            tc.tile_pool(name="const", bufs=1) as cb,
            tc.tile_pool(name="slab", bufs=slab_bufs) as slb,
            tc.tile_pool(name="sb", bufs=3) as sb,
            tc.tile_pool(name="mt", bufs=2) as mtp,
            tc.tile_pool(name="ps", bufs=2, space="PSUM") as ps,
            tc.tile_pool(name="psb", bufs=2, space="PSUM") as psb,
            tc.tile_pool(name="dram", bufs=1, space="DRAM") as dr,
        ):
            def cload(param, shape, dtype, tag):
                t = cb.tile(shape, dtype, tag=tag)
                nc.sync.dma_start(out=t[:], in_=param[:])
                return t

            iota_t = cload(iota, [P, P], bf16, "c_iota")
            w1l_t = cload(w1l, [IN_CH, HIDDEN], bf16, "c_w1l")
            w1r_t = cload(w1r, [IN_CH, HIDDEN], bf16, "c_w1r")
            w2lp_t = cload(w2lp, [HIDDEN, P], bf16, "c_w2lp")
            w2r_t = cload(w2r, [HIDDEN, OUT_CH], bf16, "c_w2r")
            b1_t = cload(b1c, [P, 1], f32, "c_b1")
            b2_t = cload(b2c, [P, 1], f32, "c_b2")
            xt_t = cload(xt, [P, nwin * P], bf16, "c_xt")
            dinv_t = cload(dinvt, [P, nwin * P], bf16, "c_dinv")
            y_t = cb.tile([OUT_CH, nwin * P], bf16, tag="c_y")
            zrow_t = cb.tile([P, P], bf16, tag="c_zrow")
            nc.vector.memset(zrow_t[:], 0.0)

            p2c = []
            for c in range(nchunk):
                p2c_t = dr.tile([max(chunk_rows[c], 1), P], bf16,
                                tag=f"p2c{c}")
                p2c.append(p2c_t)
            for c in range(nchunk):
                zr = int(p2_off[c] + NCORES * chunk_rows[c])
                nc.sync.dma_start(out=p2_full[zr:zr + 1, :], in_=zrow_t[:1, :])
                if not no_p2loc:
                    nc.sync.dma_start(out=p2_loc[zr:zr + 1, :],
                                      in_=zrow_t[:1, :])
            p2_gsrc = p2_full if no_p2loc else p2_loc

            relu = mybir.ActivationFunctionType.Relu
            copyf = mybir.ActivationFunctionType.Copy
            iseq = mybir.AluOpType.is_equal
            mult = mybir.AluOpType.mult
            add = mybir.AluOpType.add

            for _rep in range(repeat):
                # ---------------- phase 1 ----------------
                for g in range(ngrp):
                    ws = grp_windows(g)
                    a, e = grp_cols(col1, T1, g, nblk1)
                    it = sb.tile([P, slab_max // 16], i16, tag="it")
                    nc.sync.dma_start(out=it[:, :(e - a) // 16],
                                      in_=idx1[:, a // 16:e // 16])
                    dtt = sb.tile([P, slab_max // P], bf16, tag="dtt")
                    nc.sync.dma_start(out=dtt[:, :(e - a) // P],
                                      in_=dt1[:, a // P:e // P])
                    gat = slb.tile([P, slab_max], bf16, tag="g")
                    for b in range(nblk1):
                        blo = int(l1_base[b])
                        nrows = blk1_rows[b] + 1
                        # split call ranges at cell boundaries, <= maxg idxs
                        spans = []
                        for w in ws:
                            t = int(T1[w, b])
                            if t == 0:
                                continue
                            c0 = int(col1[w, b])
                            if spans and spans[-1][1] - spans[-1][0] + t <= maxg:
                                spans[-1] = (spans[-1][0], c0 + t)
                            else:
                                spans.append((c0, c0 + t))
                        for si, (ca, ce) in enumerate(spans):
                            if "nogather" in ablate:
                                continue
                            t_q = ce - ca
                            nc.gpsimd.dma_gather(
                                out_ap=gat[:, ca - a:ce - a]
                                .rearrange("p (c e) -> p c e", e=IN_CH),
                                in_ap=xdev[blo:blo + nrows, :],
                                idxs_ap=it[:, (ca - a) // 16:(ce - a) // 16],
                                num_idxs=t_q,
                                num_idxs_reg=t_q,
                                elem_size=IN_CH,
                                single_packet=single_packet,
                                queue_num=(b + si) % nq,
                            )
                    if debug and g == 0:
                        nc.sync.dma_start(out=dbg_gat[:, :4096],
                                          in_=gat[:, :4096])
                    for w in ws:
                        s1w = int(C1[w].sum())
                        n_w = min(shard - w * P, P)
                        do_mm = s1w > 0 and "nomm" not in ablate
                        if do_mm:
                            # one-hot bank for all slices of this window
                            m_t = mtp.tile([P, sw_max * P], bf16, tag="m")
                            if "noonehot" not in ablate:
                                k = 0
                                for b in range(nblk1):
                                    c0 = int(C01[w, b]) - a // P
                                    cC = int(C1[w, b])
                                    if cC == 0:
                                        continue
                                    if batch_onehot:
                                        nc.vector.tensor_tensor(
                                            out=m_t[:, k * P:(k + cC) * P]
                                            .rearrange("p (c e) -> p c e",
                                                       e=P),
                                            in0=dtt[:, c0:c0 + cC]
                                            .unsqueeze(2)
                                            .broadcast_to([P, cC, P]),
                                            in1=iota_t[:, :P]
                                            .unsqueeze(1)
                                            .broadcast_to([P, cC, P]),
                                            op=iseq)
                                        k += cC
                                    else:
                                        for sß in range(cC):
                                            nc.vector.tensor_scalar(
                                                out=m_t[:, k * P:(k + 1) * P],
                                                in0=iota_t[:],
                                                scalar1=dtt[:, c0 + sß:
                                                            c0 + sß + 1],
                                                scalar2=None, op0=iseq)
                                            k += 1
                            psum1 = ps.tile([P, P], f32, tag="psA",
                                            space="PSUM")
                            k = 0
                            for b in range(nblk1):
                                c0 = int(C01[w, b])
                                for s in range(int(C1[w, b])):
                                    sl = (c0 + s) * P - a
                                    nc.tensor.matmul(
                                        out=psum1[:],
                                        lhsT=gat[:, sl:sl + P],
                                        rhs=m_t[:, k * P:(k + 1) * P],
                                        start=(k == 0), stop=(k == s1w - 1))
                                    k += 1
                            if debug and w == 0:
                                nc.sync.dma_start(
                                    out=dbg_m[:, :min(s1w * P, 4096)],
                                    in_=m_t[:, :min(s1w * P, 4096)])
                            t1t = sb.tile([P, P], bf16, tag="t1t")
                            nc.vector.tensor_tensor(
                                out=t1t[:], in0=psum1[:],
                                in1=dinv_t[:, w * P:(w + 1) * P], op=mult)
                        else:
                            t1t = sb.tile([P, P], bf16, tag="t1t")
                            nc.vector.memset(t1t[:], 0.0)
                        psum2 = psb.tile([P, P], f32, tag="ps2", space="PSUM")
                        nc.tensor.matmul(out=psum2[:], lhsT=w1l_t[:],
                                         rhs=t1t[:], start=True, stop=False)
                        nc.tensor.matmul(out=psum2[:], lhsT=w1r_t[:],
                                         rhs=xt_t[:, w * P:(w + 1) * P],
                                         start=False, stop=True)
                        ht = sb.tile([P, P], bf16, tag="ht")
                        nc.scalar.activation(out=ht[:], in_=psum2[:],
                                             func=relu, bias=b1_t[:, :1],
                                             scale=1.0)
                        if debug:
                            nc.sync.dma_start(
                                out=dbg_t1t[:, w * P:(w + 1) * P], in_=t1t[:])
                            nc.sync.dma_start(
                                out=dbg_ht[:, w * P:(w + 1) * P], in_=ht[:])
                            nc.sync.dma_start(
                                out=dbg_dinv[:, w * P:(w + 1) * P],
                                in_=dinv_t[:, w * P:(w + 1) * P])
                            if s1w > 0:
                                raw = sb.tile([P, P], f32, tag="dbgraw")
                                nc.vector.tensor_copy(out=raw[:], in_=psum1[:])
                                nc.sync.dma_start(
                                    out=dbg_raw[:, w * P:(w + 1) * P],
                                    in_=raw[:])
                        # p2 rows [n, 128] = ht^T @ W2l_pad
                        psum3 = psb.tile([P, P], f32, tag="ps3", space="PSUM")
                        nc.tensor.matmul(out=psum3[:], lhsT=ht[:],
                                         rhs=w2lp_t[:], start=True, stop=True)
                        p2sb = sb.tile([P, P], bf16, tag="p2sb")
                        nc.scalar.activation(out=p2sb[:], in_=psum3[:],
                                             func=copyf)
                        c = min(w // chunk_wins, nchunk - 1)
                        r0 = w * P - c * chunk_wins * P
                        nc.sync.dma_start(out=p2c[c][r0:r0 + n_w, :],
                                          in_=p2sb[:n_w, :])
                        if n_w < P:
                            nc.sync.dma_start(
                                out=p2c[c][r0 + n_w:r0 + P, :],
                                in_=zrow_t[:P - n_w, :])
                        # r2 = (h @ W2r)^T + b2 -> y accumulator
                        psum4 = psb.tile([OUT_CH, P], f32, tag="ps4",
                                         space="PSUM")
                        nc.tensor.matmul(out=psum4[:], lhsT=w2r_t[:],
                                         rhs=ht[:], start=True, stop=True)
                        nc.vector.tensor_scalar(
                            out=y_t[:, w * P:(w + 1) * P], in0=psum4[:],
                            scalar1=b2_t[:OUT_CH, :1], scalar2=None, op0=add)
                        if (w + 1) % chunk_wins == 0 or w == nwin - 1:
                            c_done = (min(w // chunk_wins, nchunk - 1))
                            if (w + 1) % chunk_wins == 0 or w == nwin - 1:
                                off = int(p2_off[c_done])
                                rows = NCORES * chunk_rows[c_done]
                                if "noag" in ablate:
                                    pass
                                elif NCORES > 1:
                                    nc.gpsimd.collective_compute(
                                        "AllGather",
                                        mybir.AluOpType.bypass,
                                        replica_groups=[list(range(NCORES))],
                                        ins=[p2c[c_done].opt()],
                                        outs=[p2_full[off:off + rows, :]],
                                    )
                                else:
                                    nc.sync.dma_start(
                                        out=p2_full[off:off + rows, :],
                                        in_=p2c[c_done][:, :])
                                if "noag" not in ablate and not no_p2loc:
                                    nc.sync.dma_start(
                                        out=p2_loc[off:off + rows, :],
                                        in_=p2_full[off:off + rows, :])

                # ---------------- phase 2 ----------------
                for g in range(ngrp if "nop2" not in ablate else 0):
                    ws = grp_windows(g)
                    a, e = grp_cols(col2, T2, g, nchunk)
                    it = sb.tile([P, slab_max // 16], i16, tag="it")
                    nc.sync.dma_start(out=it[:, :(e - a) // 16],
                                      in_=idx2[:, a // 16:e // 16])
                    dtt = sb.tile([P, slab_max // P], bf16, tag="dtt")
                    nc.sync.dma_start(out=dtt[:, :(e - a) // P],
                                      in_=dt2[:, a // P:e // P])
                    gat = slb.tile([P, slab_max], bf16, tag="g")
                    for b in range(nchunk):
                        off = int(p2_off[b])
                        nrows = NCORES * chunk_rows[b] + 1
                        spans = []
                        for w in ws:
                            t = int(T2[w, b])
                            if t == 0:
                                continue
                            c0 = int(col2[w, b])
                            if spans and spans[-1][1] - spans[-1][0] + t <= maxg:
                                spans[-1] = (spans[-1][0], c0 + t)
                            else:
                                spans.append((c0, c0 + t))
                        for si, (ca, ce) in enumerate(spans):
                            if "nogather" in ablate:
                                continue
                            t_q = ce - ca
                            # queue = b % nq: chunk b's gathers wait on
                            # AllGather b; pinning them per-queue keeps a
                            # late chunk's wait from stalling the other
                            # queues' already-runnable gathers (SWDGE
                            # queues are in-order).
                            nc.gpsimd.dma_gather(
                                out_ap=gat[:, ca - a:ce - a]
                                .rearrange("p (c e) -> p c e", e=P),
                                in_ap=p2_gsrc[off:off + nrows, :],
                                idxs_ap=it[:, (ca - a) // 16:(ce - a) // 16],
                                num_idxs=t_q,
                                num_idxs_reg=t_q,
                                elem_size=P,
                                single_packet=single_packet,
                                queue_num=(b if q_by_block else b + si) % nq,
                            )
                    for w in ws:
                        s2w = int(C2[w].sum())
                        if s2w == 0 or "nomm" in ablate:
                            continue
                        m_t = mtp.tile([P, sw_max * P], bf16, tag="m")
                        if "noonehot" not in ablate:
                            k = 0
                            for b in range(nchunk):
                                c0 = int(C02[w, b]) - a // P
                                cC = int(C2[w, b])
                                if cC == 0:
                                    continue
                                if batch_onehot:
                                    nc.vector.tensor_tensor(
                                        out=m_t[:, k * P:(k + cC) * P]
                                        .rearrange("p (c e) -> p c e", e=P),
                                        in0=dtt[:, c0:c0 + cC]
                                        .unsqueeze(2)
                                        .broadcast_to([P, cC, P]),
                                        in1=iota_t[:, :P]
                                        .unsqueeze(1)
                                        .broadcast_to([P, cC, P]),
                                        op=iseq)
                                    k += cC
                                else:
                                    for sß in range(cC):
                                        nc.vector.tensor_scalar(
                                            out=m_t[:, k * P:(k + 1) * P],
                                            in0=iota_t[:],
                                            scalar1=dtt[:, c0 + sß:c0 + sß + 1],
                                            scalar2=None, op0=iseq)
                                        k += 1
                        psum5f = ps.tile([P, P], f32, tag="psA",
                                         space="PSUM")
                        psum5 = psum5f[:OUT_CH, :]
                        k = 0
                        for b in range(nchunk):
                            c0 = int(C02[w, b])
                            for s in range(int(C2[w, b])):
                                sl = (c0 + s) * P - a
                                nc.tensor.matmul(
                                    out=psum5,
                                    lhsT=gat[:, sl:sl + OUT_CH],
                                    rhs=m_t[:, k * P:(k + 1) * P],
                                    start=(k == 0), stop=(k == s2w - 1))
                                k += 1
                        agg2 = sb.tile([OUT_CH, P], bf16, tag="agg2")
                        nc.vector.tensor_tensor(
                            out=agg2[:], in0=psum5,
                            in1=dinv_t[:OUT_CH, w * P:(w + 1) * P], op=mult)
                        nc.vector.tensor_tensor(
                            out=y_t[:, w * P:(w + 1) * P], in0=agg2[:],
                            in1=y_t[:, w * P:(w + 1) * P], op=add)

            nc.sync.dma_start(out=y[:, :], in_=y_t[:, :])

    nc.compile()
    return nc


# ---------------------------------------------------------------- v4 (RS)

GRP2 = 14                    # global windows per phase-2 group


def _pack_rs(core2, w2, sl, dpos, gw, zrow):
    """Pack src-sharded phase-2 edges into per-global-window cells (128-pad).

    Single gather block (the local p2 table). Edges sorted by local src
    within each cell for gather locality. Shapes shared across cores via
    max-over-cores cell sizes.
    """
    cnt = np.bincount(core2 * gw + w2,
                      minlength=NCORES * gw).reshape(NCORES, gw)
    T = (cnt.max(axis=0) + P - 1) // P * P
    col = np.concatenate([[0], np.cumsum(T)])[:-1]
    sumT = int(T.sum())
    W0 = col // P
    SL = T // P

    order = np.lexsort((sl, w2, core2))
    co, wo, so = core2[order], w2[order], sl[order]
    keys = co.astype(np.int64) * gw + wo
    newseg = np.ones(len(keys), bool)
    newseg[1:] = keys[1:] != keys[:-1]
    idxs = np.arange(len(keys))
    starts = idxs[newseg][np.cumsum(newseg) - 1]
    rank = idxs - starts
    pos = col[wo] + rank

    # Pads reference the cell's last real row per core (see _pack); a
    # shared zero row would hotspot one HBM bank across ~100k pad descs.
    fill = np.empty((NCORES, gw), np.int64)
    fill[:, :] = (np.arange(gw)[None, :] * 997) % zrow
    fill[co, wo] = so                      # last write wins per (core, w)
    idx_flat = np.empty((NCORES, sumT), np.int32)
    for w in range(gw):
        if T[w]:
            idx_flat[:, col[w]:col[w] + T[w]] = fill[:, w][:, None]
    dt_flat = np.full((NCORES, sumT), 999.0, np.float32)
    idx_flat[co, pos] = so
    dt_flat[co, pos] = dpos[order]
    idx16 = np.ascontiguousarray(
        idx_flat.reshape(NCORES, sumT // 16, 16).transpose(0, 2, 1)
    ).astype(np.int16)
    dt = np.ascontiguousarray(
        dt_flat.reshape(NCORES, sumT // P, P).transpose(0, 2, 1))
    return idx16, dt, W0, SL, col, T, sumT


def _preprocess_rs(x, edge_index, cfg):
    import ml_dtypes
    bf16 = ml_dtypes.bfloat16
    n = x.shape[0]
    shard, nwin = cfg["shard"], cfg["nwin"]
    nblk1 = cfg["nblk1"]
    gw = NCORES * nwin

    src = np.asarray(edge_index[0], dtype=np.int64)
    dst = np.asarray(edge_index[1], dtype=np.int64)
    deg = np.bincount(dst, minlength=n).astype(np.float32)
    deg_inv = np.where(deg > 0, np.float32(1.0) / np.maximum(deg, 1.0),
                       0.0).astype(np.float32)

    # ---- phase 1: dst-sharded edges, L1 cells over 4 src blocks
    core = (dst // shard).astype(np.int64)
    local = dst - core * shard
    win = local // P
    dstl = (local % P).astype(np.int32)
    blk1 = np.minimum(src // L1_RANGE, nblk1 - 1)
    loc1 = (src - blk1 * L1_RANGE).astype(np.int32)
    blk1_rows = [min(L1_RANGE, n - q * L1_RANGE) for q in range(nblk1)]
    idx1, dt1, C01, C1, col1, T1, sumT1 = _pack(
        core, win, blk1, loc1, dstl, nwin, nblk1, blk1_rows)

    # ---- phase 2: src-sharded edges, cells by padded-global dst window
    core2 = (src // shard).astype(np.int64)
    sl = (src - core2 * shard).astype(np.int32)
    gd = (nwin * P) * (dst // shard) + (dst % shard)
    w2 = (gd // P).astype(np.int64)
    dpos = (gd % P).astype(np.int32)
    idx2, dt2, W02, SL2, col2, T2, sumT2 = _pack_rs(
        core2, w2, sl, dpos, gw, shard)

    # ---- x tables
    xb = x.astype(bf16)
    xblocks = []
    for q in range(nblk1):
        blkx = xb[q * L1_RANGE:q * L1_RANGE + blk1_rows[q]]
        xblocks.append(np.concatenate(
            [blkx, np.zeros((1, x.shape[1]), bf16)]))
    xdev = np.ascontiguousarray(np.concatenate(xblocks, axis=0))
    l1_base = np.concatenate(
        [[0], np.cumsum([b.shape[0] for b in xblocks])])[:-1]

    xts, dinvts = [], []
    for ci in range(NCORES):
        xs = x[ci * shard:(ci + 1) * shard]
        pad = nwin * P - shard
        xts.append(np.ascontiguousarray(np.concatenate(
            [xs, np.zeros((pad, x.shape[1]), np.float32)]).T).astype(bf16))
        dv = np.concatenate([deg_inv[ci * shard:(ci + 1) * shard],
                             np.zeros(pad, np.float32)])
        dinvts.append(np.ascontiguousarray(
            np.tile(dv[None, :], (P, 1))).astype(bf16))

    meta = dict(C01=C01, C1=C1, col1=col1, T1=T1, sumT1=sumT1,
                W02=W02, SL2=SL2, col2=col2, T2=T2, sumT2=sumT2,
                l1_base=l1_base, blk1_rows=blk1_rows, gw=gw)
    data = dict(xdev=xdev, idx1=idx1, dt1=dt1, idx2=idx2, dt2=dt2,
                xts=xts, dinvts=dinvts)
    return meta, data


def _build_rs(cfg, meta, repeat=1, maxg=2048, nq=4, single_packet=False,
              slab_bufs=2, slab2_bufs=2, ablate=(), batch_onehot=True):
    import concourse.bacc as bacc
    import concourse.mybir as mybir
    import concourse.tile as tile

    f32 = mybir.dt.float32
    bf16 = mybir.dt.bfloat16
    i16 = mybir.dt.int16
    shard, nwin, ngrp = cfg["shard"], cfg["nwin"], cfg["ngrp"]
    nblk1 = cfg["nblk1"]
    gw = meta["gw"]
    C01, C1, col1, T1, sumT1 = (meta[k] for k in
                                ("C01", "C1", "col1", "T1", "sumT1"))
    W02, SL2, col2, T2, sumT2 = (meta[k] for k in
                                 ("W02", "SL2", "col2", "T2", "sumT2"))
    l1_base, blk1_rows = meta["l1_base"], meta["blk1_rows"]
    xdev_rows = int(l1_base[-1] + blk1_rows[-1] + 1)
    sh_pad = nwin * P
    ngrp2 = (gw + GRP2 - 1) // GRP2

    def grp_windows(g):
        return list(range(g * GRP, min((g + 1) * GRP, nwin)))

    def grp_cols1(g):
        ws = grp_windows(g)
        a = int(col1[ws[0], 0])
        e = int(col1[ws[-1], nblk1 - 1] + T1[ws[-1], nblk1 - 1])
        return a, e

    def grp2_windows(g):
        return list(range(g * GRP2, min((g + 1) * GRP2, gw)))

    def grp2_cols(g):
        ws = grp2_windows(g)
        return int(col2[ws[0]]), int(col2[ws[-1]] + T2[ws[-1]])

    slab1_max = max(grp_cols1(g)[1] - grp_cols1(g)[0] for g in range(ngrp))
    slab2_max = max(grp2_cols(g)[1] - grp2_cols(g)[0] for g in range(ngrp2))
    s1w_max = int(C1.sum(axis=1).max())
    s2w_max = int(SL2.max())

    nc = bacc.Bacc(num_swdge_queues=nq)
    dp = nc.declare_dram_parameter
    xdev = dp("xdev", [xdev_rows, IN_CH], bf16, isOutput=False)
    xt = dp("xt", [P, sh_pad], bf16, isOutput=False)
    dinvt = dp("dinvt", [P, sh_pad], bf16, isOutput=False)
    idx1 = dp("idx1", [P, sumT1 // 16], i16, isOutput=False)
    dt1 = dp("dt1", [P, sumT1 // P], bf16, isOutput=False)
    idx2 = dp("idx2", [P, sumT2 // 16], i16, isOutput=False)
    dt2 = dp("dt2", [P, sumT2 // P], bf16, isOutput=False)
    w1l = dp("w1l", [IN_CH, HIDDEN], bf16, isOutput=False)
    w1r = dp("w1r", [IN_CH, HIDDEN], bf16, isOutput=False)
    w2lp = dp("w2lp", [HIDDEN, P], bf16, isOutput=False)
    w2r = dp("w2r", [HIDDEN, OUT_CH], bf16, isOutput=False)
    b1c = dp("b1c", [P, 1], f32, isOutput=False)
    b2c = dp("b2c", [P, 1], f32, isOutput=False)
    iota = dp("iota", [P, P], bf16, isOutput=False)
    y = dp("y", [OUT_CH, sh_pad], bf16, isOutput=True)

    p2l = nc.dram_tensor("p2l", [shard + 1, P], bf16)
    # ch-major partials: core block cb occupies rows [cb*64, (cb+1)*64) —
    # flat-contiguous per block, so ReduceScatter hands core cb its own
    # [64, sh_pad] slice already in y_t orientation (no transposes).
    part = nc.dram_tensor("part", [NCORES * OUT_CH, sh_pad], bf16)
    rs_out = nc.dram_tensor("rs_out", [OUT_CH, sh_pad], bf16)
    grp_per_blk = nwin // GRP2
    assert nwin % GRP2 == 0, "phase-2 groups must not straddle core blocks"

    with tile.TileContext(nc) as tc:
        with (
            tc.tile_pool(name="const", bufs=1) as cb,
            tc.tile_pool(name="slab", bufs=slab_bufs) as slb,
            tc.tile_pool(name="slab2", bufs=slab2_bufs) as slb2,
            tc.tile_pool(name="sb", bufs=3) as sb,
            tc.tile_pool(name="mt", bufs=2) as mtp,
            tc.tile_pool(name="pg", bufs=2) as pgp,
            tc.tile_pool(name="ps", bufs=2, space="PSUM") as ps,
            tc.tile_pool(name="psb", bufs=2, space="PSUM") as psb,
        ):
            def cload(param, shape, dtype, tag):
                t = cb.tile(shape, dtype, tag=tag)
                nc.sync.dma_start(out=t[:], in_=param[:])
                return t

            iota_t = cload(iota, [P, P], bf16, "c_iota")
            w1l_t = cload(w1l, [IN_CH, HIDDEN], bf16, "c_w1l")
            w1r_t = cload(w1r, [IN_CH, HIDDEN], bf16, "c_w1r")
            w2lp_t = cload(w2lp, [HIDDEN, P], bf16, "c_w2lp")
            w2r_t = cload(w2r, [HIDDEN, OUT_CH], bf16, "c_w2r")
            b1_t = cload(b1c, [P, 1], f32, "c_b1")
            b2_t = cload(b2c, [P, 1], f32, "c_b2")
            xt_t = cload(xt, [P, sh_pad], bf16, "c_xt")
            dinv_t = cload(dinvt, [P, sh_pad], bf16, "c_dinv")
            y_t = cb.tile([OUT_CH, sh_pad], bf16, tag="c_y")
            zrow_t = cb.tile([P, P], bf16, tag="c_zrow")
            nc.vector.memset(zrow_t[:], 0.0)
            nc.sync.dma_start(out=p2l[shard:shard + 1, :], in_=zrow_t[:1, :])

            relu = mybir.ActivationFunctionType.Relu
            copyf = mybir.ActivationFunctionType.Copy
            iseq = mybir.AluOpType.is_equal
            mult = mybir.AluOpType.mult
            add = mybir.AluOpType.add

            for _rep in range(repeat):
                # ---------------- phase 1 (dst-sharded, as v2) ----------
                for g in range(ngrp):
                    ws = grp_windows(g)
                    a, e = grp_cols1(g)
                    it = sb.tile([P, slab1_max // 16], i16, tag="it")
                    nc.sync.dma_start(out=it[:, :(e - a) // 16],
                                      in_=idx1[:, a // 16:e // 16])
                    dtt = sb.tile([P, slab1_max // P], bf16, tag="dtt")
                    nc.sync.dma_start(out=dtt[:, :(e - a) // P],
                                      in_=dt1[:, a // P:e // P])
                    gat = slb.tile([P, slab1_max], bf16, tag="g")
                    for b in range(nblk1):
                        blo = int(l1_base[b])
                        nrows = blk1_rows[b] + 1
                        spans = []
                        for w in ws:
                            t = int(T1[w, b])
                            if t == 0:
                                continue
                            c0 = int(col1[w, b])
                            if spans and spans[-1][1] - spans[-1][0] + t <= maxg:
                                spans[-1] = (spans[-1][0], c0 + t)
                            else:
                                spans.append((c0, c0 + t))
                        for si, (ca, ce) in enumerate(spans):
                            if "nogather" in ablate:
                                continue
                            t_q = ce - ca
                            nc.gpsimd.dma_gather(
                                out_ap=gat[:, ca - a:ce - a]
                                .rearrange("p (c e) -> p c e", e=IN_CH),
                                in_ap=xdev[blo:blo + nrows, :],
                                idxs_ap=it[:, (ca - a) // 16:(ce - a) // 16],
                                num_idxs=t_q,
                                num_idxs_reg=t_q,
                                elem_size=IN_CH,
                                single_packet=single_packet,
                                queue_num=(b + si) % nq,
                            )
                    for w in ws:
                        s1w = int(C1[w].sum())
                        n_w = min(shard - w * P, P)
                        if s1w > 0:
                            m_t = mtp.tile([P, s1w_max * P], bf16, tag="m")
                            k = 0
                            for b in range(nblk1):
                                c0 = int(C01[w, b]) - a // P
                                cC = int(C1[w, b])
                                if cC == 0:
                                    continue
                                if batch_onehot:
                                    nc.vector.tensor_tensor(
                                        out=m_t[:, k * P:(k + cC) * P]
                                        .rearrange("p (c e) -> p c e", e=P),
                                        in0=dtt[:, c0:c0 + cC]
                                        .unsqueeze(2)
                                        .broadcast_to([P, cC, P]),
                                        in1=iota_t[:, :P]
                                        .unsqueeze(1)
                                        .broadcast_to([P, cC, P]),
                                        op=iseq)
                                    k += cC
                                else:
                                    for s in range(cC):
                                        nc.vector.tensor_scalar(
                                            out=m_t[:, k * P:(k + 1) * P],
                                            in0=iota_t[:],
                                            scalar1=dtt[:, c0 + s:c0 + s + 1],
                                            scalar2=None, op0=iseq)
                                        k += 1
                            psum1 = ps.tile([P, P], f32, tag="psA",
                                            space="PSUM")
                            k = 0
                            for b in range(nblk1):
                                c0 = int(C01[w, b])
                                for s in range(int(C1[w, b])):
                                    sl_ = (c0 + s) * P - a
                                    nc.tensor.matmul(
                                        out=psum1[:],
                                        lhsT=gat[:, sl_:sl_ + P],
                                        rhs=m_t[:, k * P:(k + 1) * P],
                                        start=(k == 0), stop=(k == s1w - 1))
                                    k += 1
                            t1t = sb.tile([P, P], bf16, tag="t1t")
                            nc.vector.tensor_tensor(
                                out=t1t[:], in0=psum1[:],
                                in1=dinv_t[:, w * P:(w + 1) * P], op=mult)
                        else:
                            t1t = sb.tile([P, P], bf16, tag="t1t")
                            nc.vector.memset(t1t[:], 0.0)
                        psum2 = psb.tile([P, P], f32, tag="ps2", space="PSUM")
                        nc.tensor.matmul(out=psum2[:], lhsT=w1l_t[:],
                                         rhs=t1t[:], start=True, stop=False)
                        nc.tensor.matmul(out=psum2[:], lhsT=w1r_t[:],
                                         rhs=xt_t[:, w * P:(w + 1) * P],
                                         start=False, stop=True)
                        ht = sb.tile([P, P], bf16, tag="ht")
                        nc.scalar.activation(out=ht[:], in_=psum2[:],
                                             func=relu, bias=b1_t[:, :1],
                                             scale=1.0)
                        # p2 rows -> local table
                        psum3 = psb.tile([P, P], f32, tag="ps3", space="PSUM")
                        nc.tensor.matmul(out=psum3[:], lhsT=ht[:],
                                         rhs=w2lp_t[:], start=True, stop=True)
                        p2sb = sb.tile([P, P], bf16, tag="p2sb")
                        nc.scalar.activation(out=p2sb[:], in_=psum3[:],
                                             func=copyf)
                        nc.sync.dma_start(out=p2l[w * P:w * P + n_w, :],
                                          in_=p2sb[:n_w, :])
                        # root term -> y accumulator
                        psum4 = psb.tile([OUT_CH, P], f32, tag="ps4",
                                         space="PSUM")
                        nc.tensor.matmul(out=psum4[:], lhsT=w2r_t[:],
                                         rhs=ht[:], start=True, stop=True)
                        nc.vector.tensor_scalar(
                            out=y_t[:, w * P:(w + 1) * P], in0=psum4[:],
                            scalar1=b2_t[:OUT_CH, :1], scalar2=None, op0=add)

                # ---------------- phase 2 (src-sharded partials) --------
                for g in range(ngrp2 if "nop2" not in ablate else 0):
                    ws2 = grp2_windows(g)
                    a, e = grp2_cols(g)
                    it = sb.tile([P, slab2_max // 16], i16, tag="it2")
                    nc.sync.dma_start(out=it[:, :(e - a) // 16],
                                      in_=idx2[:, a // 16:e // 16])
                    dtt = sb.tile([P, slab2_max // P], bf16, tag="dtt2")
                    nc.sync.dma_start(out=dtt[:, :(e - a) // P],
                                      in_=dt2[:, a // P:e // P])
                    gat = slb2.tile([P, slab2_max], bf16, tag="g2")
                    spans = []
                    for w in ws2:
                        t = int(T2[w])
                        if t == 0:
                            continue
                        c0 = int(col2[w])
                        if spans and spans[-1][1] - spans[-1][0] + t <= maxg:
                            spans[-1] = (spans[-1][0], c0 + t)
                        else:
                            spans.append((c0, c0 + t))
                    for si, (ca, ce) in enumerate(spans):
                        if "nogather" in ablate:
                            continue
                        t_q = ce - ca
                        nc.gpsimd.dma_gather(
                            out_ap=gat[:, ca - a:ce - a]
                            .rearrange("p (c e) -> p c e", e=P),
                            in_ap=p2l[:, :],
                            idxs_ap=it[:, (ca - a) // 16:(ce - a) // 16],
                            num_idxs=t_q,
                            num_idxs_reg=t_q,
                            elem_size=P,
                            single_packet=single_packet,
                            queue_num=si % nq,
                        )
                    pg = pgp.tile([OUT_CH, GRP2 * P], bf16, tag="pg")
                    for wi, w in enumerate(ws2):
                        s2w = int(SL2[w])
                        if s2w == 0:
                            nc.vector.memset(
                                pg[:, wi * P:(wi + 1) * P], 0.0)
                            continue
                        m_t = mtp.tile([P, s2w_max * P], bf16, tag="m2")
                        c0 = int(W02[w]) - a // P
                        nc.vector.tensor_tensor(
                            out=m_t[:, :s2w * P]
                            .rearrange("p (c e) -> p c e", e=P),
                            in0=dtt[:, c0:c0 + s2w]
                            .unsqueeze(2).broadcast_to([P, s2w, P]),
                            in1=iota_t[:, :P]
                            .unsqueeze(1).broadcast_to([P, s2w, P]),
                            op=iseq)
                        psum5f = ps.tile([P, P], f32, tag="psA",
                                         space="PSUM")
                        psum5 = psum5f[:OUT_CH, :]
                        for s in range(s2w):
                            sl_ = (int(W02[w]) + s) * P - a
                            nc.tensor.matmul(
                                out=psum5,
                                lhsT=gat[:, sl_:sl_ + OUT_CH],
                                rhs=m_t[:, s * P:(s + 1) * P],
                                start=(s == 0), stop=(s == s2w - 1))
                        nc.scalar.activation(
                            out=pg[:, wi * P:(wi + 1) * P],
                            in_=psum5, func=copyf)
                    if "nopart" not in ablate:
                        cb = g // grp_per_blk
                        lw0 = (g % grp_per_blk) * GRP2
                        nc.sync.dma_start(
                            out=part[cb * OUT_CH:(cb + 1) * OUT_CH,
                                     lw0 * P:lw0 * P + len(ws2) * P],
                            in_=pg[:, :len(ws2) * P])

                # ---------------- ReduceScatter + combine ---------------
                if "nop2" not in ablate and "nocomb" not in ablate:
                    if "nors" not in ablate:
                        nc.gpsimd.collective_compute(
                            "ReduceScatter",
                            mybir.AluOpType.add,
                            replica_groups=[list(range(NCORES))],
                            ins=[part[:, :]],
                            outs=[rs_out[:, :]],
                        )
                    for g in range(ngrp):
                        ws = grp_windows(g)
                        r0 = ws[0] * P
                        nr = len(ws) * P
                        rsc = sb.tile([OUT_CH, GRP * P], bf16, tag="rsc")
                        nc.sync.dma_start(out=rsc[:, :nr],
                                          in_=rs_out[:, r0:r0 + nr])
                        agg = sb.tile([OUT_CH, GRP * P], bf16, tag="aggg")
                        nc.vector.tensor_tensor(
                            out=agg[:, :nr], in0=rsc[:, :nr],
                            in1=dinv_t[:OUT_CH, r0:r0 + nr], op=mult)
                        nc.vector.tensor_tensor(
                            out=y_t[:, r0:r0 + nr],
                            in0=agg[:, :nr],
                            in1=y_t[:, r0:r0 + nr], op=add)

            nc.sync.dma_start(out=y[:, :], in_=y_t[:, :])

    nc.compile()
    return nc


def _make_inmaps_rs(inputs, meta, data):
    import ml_dtypes
    bf16 = ml_dtypes.bfloat16
    iota_v = np.tile(np.arange(P, dtype=np.float32), (P, 1)).astype(bf16)
    id_v = np.eye(P, dtype=np.float32)
    w2l = np.asarray(inputs["W2_l"], np.float32)
    w2lp = np.concatenate(
        [w2l, np.zeros((HIDDEN, P - OUT_CH), np.float32)], axis=1)
    b2 = np.asarray(inputs["b2"], np.float32)
    common = dict(
        xdev=data["xdev"],
        w1l=np.asarray(inputs["W1_l"], np.float32).astype(bf16),
        w1r=np.asarray(inputs["W1_r"], np.float32).astype(bf16),
        w2lp=w2lp.astype(bf16),
        w2r=np.asarray(inputs["W2_r"], np.float32).astype(bf16),
        b1c=np.asarray(inputs["b1"], np.float32).reshape(P, 1),
        b2c=np.concatenate([b2, np.zeros(P - OUT_CH, np.float32)]
                           ).reshape(P, 1),
        iota=iota_v,
        id128=id_v.astype(bf16),
    )
    in_maps = []
    for ci in range(NCORES):
        m = dict(common)
        m["xt"] = data["xts"][ci]
        m["dinvt"] = data["dinvts"][ci]
        m["idx1"] = np.tile(data["idx1"][ci], (8, 1))
        m["dt1"] = data["dt1"][ci].astype(bf16)
        m["idx2"] = np.tile(data["idx2"][ci], (8, 1))
        m["dt2"] = data["dt2"][ci].astype(bf16)
        in_maps.append(m)
    return in_maps


# ---------------------------------------------------------------- entry

_CACHE = {}


def _meta_sig(meta):
    return (int(meta["sumT1"]), int(meta["sumT2"]))


def _make_inmaps(inputs, meta, data):
    import ml_dtypes
    bf16 = ml_dtypes.bfloat16
    iota_v = np.tile(np.arange(P, dtype=np.float32),
                     (P, 1)).astype(bf16)
    w2l = np.asarray(inputs["W2_l"], np.float32)
    w2lp = np.concatenate(
        [w2l, np.zeros((HIDDEN, P - OUT_CH), np.float32)], axis=1)
    b2 = np.asarray(inputs["b2"], np.float32)
    common = dict(
        xdev=data["xdev"],
        w1l=np.asarray(inputs["W1_l"], np.float32).astype(bf16),
        w1r=np.asarray(inputs["W1_r"], np.float32).astype(bf16),
        w2lp=w2lp.astype(bf16),
        w2r=np.asarray(inputs["W2_r"], np.float32).astype(bf16),
        b1c=np.asarray(inputs["b1"], np.float32).reshape(P, 1),
        b2c=np.concatenate([b2, np.zeros(P - OUT_CH, np.float32)]
                           ).reshape(P, 1),
        iota=iota_v,
    )
    in_maps = []
    for ci in range(NCORES):
        m = dict(common)
        m["xt"] = data["xts"][ci]
        m["dinvt"] = data["dinvts"][ci]
        m["idx1"] = np.tile(data["idx1"][ci], (8, 1))
        m["dt1"] = data["dt1"][ci].astype(bf16)
        m["idx2"] = np.tile(data["idx2"][ci], (8, 1))
        m["dt2"] = data["dt2"][ci].astype(bf16)
        in_maps.append(m)
    return in_maps


# Which kernel strategy kernel() uses: "rs" (src-sharded phase 2 +
# ReduceScatter, v4) or "ag" (dst-sharded + AllGather, v2/v3lite).
# HW-measured (within-batch differentials): ag/maxg3072 ~1.25 ms/iter,
# ag/maxg2048 ~1.28, rs best ~1.32. q_by_block measured slower; keep off.
KERNEL_KIND = "ag"
BUILD_KW = dict(maxg=1024)
BUILD_KW_AG = dict(maxg=3072, no_p2loc=True, batch_onehot=True)


def _prep_any(x, edge_index, cfg):
    if KERNEL_KIND == "rs":
        return _preprocess_rs(x, edge_index, cfg)
    # sort_loc: ascending source rows within each gather call measured
    # ~58 us/iter faster than input order (better HBM row locality).
    return _preprocess(x, edge_index, cfg, sort_loc=True)


def _build_any(cfg, meta, repeat=1, **kw):
    if KERNEL_KIND == "rs":
        return _build_rs(cfg, meta, repeat=repeat, **{**BUILD_KW, **kw})
    return _build(cfg, meta, repeat=repeat, **{**BUILD_KW_AG, **kw})


def _inmaps_any(inputs, meta, data):
    if KERNEL_KIND == "rs":
        return _make_inmaps_rs(inputs, meta, data)
    return _make_inmaps(inputs, meta, data)


def kernel(x, edge_index, W1_l, W1_r, b1, W2_l, W2_r, b2):
    x = np.asarray(x, dtype=np.float32)
    edge_index = np.asarray(edge_index)
    cfg = _derive_cfg(x.shape[0])
    meta, data = _prep_any(x, edge_index, cfg)

    key = (KERNEL_KIND, x.shape, edge_index.shape)
    if key in _CACHE and _CACHE[key][1] == _meta_sig(meta):
        nc = _CACHE[key][0]
    else:
        nc = _build_any(cfg, meta)
        _CACHE[key] = (nc, _meta_sig(meta))

    in_maps = _inmaps_any(
        dict(W1_l=W1_l, W1_r=W1_r, b1=b1, W2_l=W2_l, W2_r=W2_r, b2=b2),
        meta, data)

    from concourse.bass_utils import run_bass_kernel_spmd
    r = run_bass_kernel_spmd(nc, in_maps, core_ids=list(range(NCORES)))
    shard = cfg["shard"]
    out = np.concatenate(
        [r.results[c]["y"].astype(np.float32).T[:shard]
         for c in range(NCORES)], axis=0)
    return np.ascontiguousarray(out, dtype=np.float32)

